# revision 1
# baseline (speedup 1.0000x reference)
"""Bidirectional linear attention kernel for 8 TRN2 NeuronCores.

Sharding: core i handles batch b = i//2, token half i%2 (4096 tokens each).
Per-head KV aggregation is completed with an AllReduce of the packed
[kv | ksum] accumulator (528KB) within core pairs {0,1},{2,3},{4,5},{6,7}.

Layout strategy (per core):
  - x arrives host-transposed as xT [D, Ntok] (feature-major).
  - Phase 1: k, v computed token-major ([tok, D]) with fp32r matmuls;
    phi(k) = max(k+1, exp(min(k,0))) fused ACT+DVE; kv per head-pair via
    bf16 matmuls with a ones-column appended to v so ksum falls out of the
    same matmul (out [128, 129] per pair), accumulated into SBUF.
  - AllReduce pairs over the packed [128, 1032] kv/ksum buffer.
  - Phase 2: qT computed feature-major; numerator^T = blockdiag(kv)^T @ qT;
    denominator for all heads via blockdiag-ksum matmul -> [16, T];
    reciprocal broadcast back to 64 partitions/head with a small select
    matmul; divide; output projection token-major + bias.
"""

import os
import sys

import numpy as np

for _p in ("/opt/trn_rl_repo", "/root/.axon_site/_ro/trn_rl_repo"):
    if os.path.isdir(_p) and _p not in sys.path:
        sys.path.append(_p)

from contextlib import ExitStack

import concourse.bacc as bacc
import concourse.tile as tile
from concourse import mybir
from concourse.alu_op_type import AluOpType
from concourse import bass_utils

F32 = mybir.dt.float32
F32R = mybir.dt.float32r
BF16 = mybir.dt.bfloat16
AF = mybir.ActivationFunctionType

D = 1024        # model dim
H = 16          # heads
HD = 64         # head dim
P = 128         # partitions
NPAIR = 8       # head pairs
KC = 8          # feature chunks of 128
TC = 512        # token chunk
EPS = 1e-6
PW = 129        # pair width in the packed kv|ksum layout


def r(ap):
    return ap.bitcast(F32R)


def build_program(nc, ntok, use_cc=True, reps=1, mmdt="f32r",
                  do_p1=True, do_p2=True, abl=(), fuseq=False, rbc_gps=False,
                  atbf=False, qcp=False, kcp=False):
    abl = set(abl)
    nch = ntok // TC
    MDT = F32R if mmdt == "f32r" else BF16
    XDT = F32 if mmdt == "f32r" else BF16

    def rr(ap):
        return ap.bitcast(F32R) if mmdt == "f32r" else ap

    xt = nc.dram_tensor("xt", [D, ntok], XDT, kind="ExternalInput").ap()
    wqt = nc.dram_tensor("wqt", [D, D], XDT, kind="ExternalInput").ap()
    wkt = nc.dram_tensor("wkt", [D, D], XDT, kind="ExternalInput").ap()
    wvt = nc.dram_tensor("wvt", [D, D], XDT, kind="ExternalInput").ap()
    wot = (nc.dram_tensor("wotb", [D, D], BF16, kind="ExternalInput").ap()
           if atbf else
           nc.dram_tensor("wot", [D, D], XDT, kind="ExternalInput").ap())
    bias = nc.dram_tensor("bias", [1, D], F32, kind="ExternalInput").ap()
    out = nc.dram_tensor("out", [ntok, D], F32, kind="ExternalOutput").ap()

    cc_in = nc.dram_tensor("cc_in", [P, NPAIR * PW], F32).ap()
    cc_out = nc.dram_tensor("cc_out", [P, NPAIR * PW], F32).ap()
    QDT = BF16 if fuseq else MDT
    qsp = nc.dram_tensor("qsp", [D, ntok], BF16).ap() if fuseq else None

    with tile.TileContext(nc) as tc, ExitStack() as es:
        p_w = es.enter_context(tc.tile_pool(name="w", bufs=24 if fuseq else 16))
        p_x = es.enter_context(tc.tile_pool(name="x", bufs=9))
        p_scr = es.enter_context(tc.tile_pool(name="scr", bufs=3 if atbf else 2))
        p_qc = es.enter_context(tc.tile_pool(name="qc", bufs=3))
        p_c1 = es.enter_context(tc.tile_pool(name="c1", bufs=1))
        p_c8 = es.enter_context(tc.tile_pool(name="c8", bufs=8))

        # bias broadcast [1, D] -> [128, D]
        b_row = p_c1.tile([1, D], F32, tag="brow")
        nc.sync.dma_start(b_row[:], bias[:, :])
        bias_bc = p_c1.tile([P, D], F32, tag="bias")
        nc.gpsimd.partition_broadcast(bias_bc[:], b_row[:])

        # select matrices for the reciprocal broadcast (host-prepared constant)
        selc = nc.dram_tensor("selcb" if fuseq else "selc", [H, NPAIR * P],
                          BF16 if fuseq else XDT, kind="ExternalInput").ap()
        sel_t = p_c1.tile([H, NPAIR * P], QDT, tag="sel")
        nc.sync.dma_start(sel_t[:], selc[:, :] if fuseq else rr(selc[:, :]))
        sel = [sel_t[:, p * P:(p + 1) * P] for p in range(NPAIR)]

        # masks for block-diag kv / ksum lhsT construction
        dmask = nc.dram_tensor("dmask", [P, P], F32, kind="ExternalInput").ap()
        dmask_t = p_c1.tile([P, P], F32, tag="dmask")
        nc.sync.dma_start(dmask_t[:], dmask[:, :])
        kmask = nc.dram_tensor("kmask", [P, KC * H], F32,
                               kind="ExternalInput").ap()
        kmask_t = p_c1.tile([P, KC * H], F32, tag="kmask")
        nc.sync.dma_start(kmask_t[:], kmask[:, :])

        # packed kv | ksum accumulator
        kvks = p_c1.tile([P, NPAIR * PW], F32, tag="kvks")
        kvks3 = kvks.rearrange("p (a b) -> p a b", b=PW)

        def load_xt(j):
            ts = []
            for k in range(KC):
                t = p_x.tile([P, TC], MDT, tag="xt")
                nc.sync.dma_start(
                    t[:], rr(xt[k * P:(k + 1) * P, j * TC:(j + 1) * TC]))
                ts.append(t)
            return ts

        for _rep in range(reps):
            # ---- weights (phase 1) ----
            wkt_t = []
            wvt_t = []
            wq1_t = []
            if do_p1:
                for k in range(KC):
                    t = p_w.tile([P, D], MDT, tag="w")
                    nc.sync.dma_start(t[:], rr(wkt[k * P:(k + 1) * P, :]))
                    wkt_t.append(t)
                for k in range(KC):
                    t = p_w.tile([P, D], MDT, tag="w")
                    nc.sync.dma_start(t[:], rr(wvt[k * P:(k + 1) * P, :]))
                    wvt_t.append(t)
                if fuseq:
                    for k in range(KC):
                        t = p_w.tile([P, D], MDT, tag="w")
                        nc.sync.dma_start(t[:], rr(wqt[k * P:(k + 1) * P, :]))
                        wq1_t.append(t)
            nc.vector.memset(kvks[:], 0.0)

            # ================= phase 1: k, v, kv, ksum =================
            with ExitStack() as es1:
                ps_k = es1.enter_context(tc.tile_pool(name="psk", bufs=2, space="PSUM"))
                ps_v = es1.enter_context(tc.tile_pool(name="psv", bufs=2, space="PSUM"))
                ps_kv = es1.enter_context(tc.tile_pool(name="pskv", bufs=2, space="PSUM"))
                p_kphi = es1.enter_context(tc.tile_pool(name="kphi", bufs=6))
                p_v = es1.enter_context(tc.tile_pool(name="vsb", bufs=6))
                if fuseq:
                    ps_q1 = es1.enter_context(
                        tc.tile_pool(name="psq1", bufs=2, space="PSUM"))
                    p_q1 = es1.enter_context(tc.tile_pool(name="q1", bufs=3))

                for j in range(nch if do_p1 else 0):
                    xts = load_xt(j)
                    kphis = []
                    vsbs = []
                    for m in range(4):
                        xm = [xts[k][:, m * P:(m + 1) * P] for k in range(KC)]
                        # k projection + phi -> bf16 [128, 1024]
                        kph = p_kphi.tile([P, D], BF16, tag="kphi")
                        for n in range(2):
                            kp = ps_k.tile([P, TC], F32, tag="kp")
                            for k in range(KC):
                                nc.tensor.matmul(
                                    kp[:], xm[k], wkt_t[k][:, n * TC:(n + 1) * TC],
                                    start=(k == 0), stop=(k == KC - 1))
                            if "phi" in abl:
                                nc.scalar.activation(
                                    kph[:, n * TC:(n + 1) * TC], kp[:], AF.Copy)
                            elif kcp:
                                kc_ = p_qc.tile([P, TC], F32, tag="kc")
                                nc.scalar.activation(kc_[:], kp[:], AF.Copy)
                                mx = p_scr.tile([P, TC], F32, tag="mx")
                                nc.vector.tensor_scalar_min(mx[:], kc_[:], 0.0)
                                ex = p_scr.tile([P, TC], F32, tag="ex")
                                nc.scalar.activation(ex[:], mx[:], AF.Exp)
                                nc.vector.scalar_tensor_tensor(
                                    kph[:, n * TC:(n + 1) * TC], kc_[:], 1.0,
                                    ex[:], AluOpType.add, AluOpType.max)
                            else:
                                mx = p_scr.tile([P, TC], F32, tag="mx")
                                nc.vector.tensor_scalar_min(mx[:], kp[:], 0.0)
                                ex = p_scr.tile([P, TC], F32, tag="ex")
                                nc.scalar.activation(ex[:], mx[:], AF.Exp)
                                nc.vector.scalar_tensor_tensor(
                                    kph[:, n * TC:(n + 1) * TC], kp[:], 1.0, ex[:],
                                    AluOpType.add, AluOpType.max)
                        kphis.append(kph)
                        # v projection -> bf16 [128, 1032] interleaved with ones cols
                        vsb = p_v.tile([P, NPAIR * PW], BF16, tag="vsb")
                        vsb3 = vsb.rearrange("p (a b) -> p a b", b=PW)
                        for n in range(2):
                            vp = ps_v.tile([P, TC], F32, tag="vp")
                            for k in range(KC):
                                nc.tensor.matmul(
                                    vp[:], xm[k], wvt_t[k][:, n * TC:(n + 1) * TC],
                                    start=(k == 0), stop=(k == KC - 1))
                            if "vcopy" not in abl:
                                nc.scalar.activation(
                                    vsb3[:, 4 * n:4 * n + 4, 0:P],
                                    vp.rearrange("p (a b) -> p a b", b=P), AF.Copy)
                        if "vcopy" not in abl:
                            nc.vector.memset(vsb3[:, :, P:PW], 1.0)
                        else:
                            nc.vector.memset(vsb[:], 1.0)
                        vsbs.append(vsb)
                    # fused q projection + phi + spill to DRAM (bf16)
                    if fuseq:
                        for c in range(KC):
                            qp1 = ps_q1.tile([P, TC], F32, tag="qp1")
                            for k in range(KC):
                                nc.tensor.matmul(
                                    qp1[:], wq1_t[k][:, c * P:(c + 1) * P],
                                    xts[k][:],
                                    start=(k == 0), stop=(k == KC - 1))
                            q1 = p_q1.tile([P, TC], BF16, tag="q1")
                            if "phi" in abl:
                                nc.scalar.activation(q1[:], qp1[:], AF.Copy)
                            else:
                                mx = p_scr.tile([P, TC], F32, tag="mx")
                                nc.vector.tensor_scalar_min(mx[:], qp1[:], 0.0)
                                ex = p_scr.tile([P, TC], F32, tag="ex")
                                nc.scalar.activation(ex[:], mx[:], AF.Exp)
                                nc.vector.scalar_tensor_tensor(
                                    q1[:], qp1[:], 1.0, ex[:], AluOpType.add,
                                    AluOpType.max)
                            nc.sync.dma_start(
                                qsp[c * P:(c + 1) * P, j * TC:(j + 1) * TC],
                                q1[:])
                    # kv + ksum accumulation per pair
                    for p in range(NPAIR if "kv" not in abl else 0):
                        kvp = ps_kv.tile([P, PW], F32, tag="kvp")
                        for m in range(4):
                            nc.tensor.matmul(
                                kvp[:], kphis[m][:, p * P:(p + 1) * P],
                                vsbs[m].rearrange("p (a b) -> p a b", b=PW)[:, p, :],
                                start=(m == 0), stop=(m == 3))
                        nc.vector.tensor_tensor(
                            kvks3[:, p, :], kvp[:], kvks3[:, p, :], AluOpType.add)

            # ================= allreduce within pairs =================
            nc.sync.dma_start(cc_in[:, :], kvks[:])
            if use_cc:
                nc.gpsimd.collective_compute(
                    "AllReduce", AluOpType.add,
                    replica_groups=[[0, 1], [2, 3], [4, 5], [6, 7]],
                    ins=[cc_in[:, :]], outs=[cc_out[:, :]])
            else:
                nc.sync.dma_start(cc_out[:, :], cc_in[:, :])
            red = p_c1.tile([P, NPAIR * PW], F32, tag="red")
            nc.sync.dma_start(red[:], cc_out[:, :])
            red3 = red.rearrange("p (a b) -> p a b", b=PW)

            # block-diag kv lhsT per pair (diag-mask multiply), block ksum lhsT
            # per chunk (per-partition scalar multiply against a column mask).
            kvbd = []
            ksbd = []
            for p in range(NPAIR):
                t = p_c8.tile([P, P], QDT, tag="kvbd")
                nc.vector.tensor_tensor(t[:], red3[:, p, 0:P], dmask_t[:],
                                        AluOpType.mult)
                kvbd.append(t)
            for c in range(KC):
                t = p_c8.tile([P, H], QDT, tag="ksbd")
                nc.vector.tensor_scalar(
                    t[:], kmask_t[:, c * H:(c + 1) * H], red3[:, c, P:PW], None,
                    AluOpType.mult)
                ksbd.append(t)

            # ---- weights (phase 2) ----
            wqt_t = []
            wot_t = []
            if do_p2:
                for k in range(KC):
                    t = p_w.tile([P, D], MDT, tag="w")
                    nc.sync.dma_start(t[:], rr(wqt[k * P:(k + 1) * P, :]))
                    wqt_t.append(t)
                for k in range(KC):
                    if atbf:
                        t = p_w.tile([P, D], BF16, tag="w")
                        nc.sync.dma_start(t[:], wot[k * P:(k + 1) * P, :])
                    else:
                        t = p_w.tile([P, D], MDT, tag="w")
                        nc.sync.dma_start(t[:], rr(wot[k * P:(k + 1) * P, :]))
                    wot_t.append(t)

            # ================= phase 2: q, numerator, denom, y =================
            with ExitStack() as es2:
                ps_q = es2.enter_context(tc.tile_pool(name="psq", bufs=2, space="PSUM"))
                ps_num = es2.enter_context(tc.tile_pool(name="psnum", bufs=2, space="PSUM"))
                ps_rbc = es2.enter_context(tc.tile_pool(name="psrbc", bufs=2, space="PSUM"))
                ps_y = es2.enter_context(tc.tile_pool(name="psy", bufs=2, space="PSUM"))
                p_qt = es2.enter_context(tc.tile_pool(name="qt", bufs=16))
                p_at = es2.enter_context(tc.tile_pool(name="at", bufs=16 if atbf else 11))
                p_rbc = es2.enter_context(tc.tile_pool(name="rbc", bufs=3))
                p_y = es2.enter_context(tc.tile_pool(name="ysb", bufs=3))
                p_dn = es2.enter_context(tc.tile_pool(name="dn", bufs=1))

                def emit_qproj(j):
                    xts = load_xt(j)
                    qts = []
                    for c in range(KC):
                        qp = ps_q.tile([P, TC], F32, tag="qp")
                        for k in range(KC):
                            nc.tensor.matmul(
                                qp[:], wqt_t[k][:, c * P:(c + 1) * P], xts[k][:],
                                start=(k == 0), stop=(k == KC - 1))
                        qt_ = p_qt.tile([P, TC], MDT, tag="qt")
                        if "phi" in abl:
                            nc.scalar.activation(qt_[:], qp[:], AF.Copy)
                        elif qcp:
                            # release the PSUM tile with one ACT copy, then
                            # run the phi chain from SBUF so the PE is not
                            # gated on the full DVE/ACT consumer chain
                            qc = p_qc.tile([P, TC], F32, tag="qc")
                            nc.scalar.activation(qc[:], qp[:], AF.Copy)
                            mx = p_scr.tile([P, TC], F32, tag="mx")
                            nc.vector.tensor_scalar_min(mx[:], qc[:], 0.0)
                            ex = p_scr.tile([P, TC], F32, tag="ex")
                            nc.scalar.activation(ex[:], mx[:], AF.Exp)
                            nc.vector.scalar_tensor_tensor(
                                qt_[:], qc[:], 1.0, ex[:], AluOpType.add,
                                AluOpType.max)
                        else:
                            mx = p_scr.tile([P, TC], F32, tag="mx")
                            nc.vector.tensor_scalar_min(mx[:], qp[:], 0.0)
                            ex = p_scr.tile([P, TC], F32, tag="ex")
                            nc.scalar.activation(ex[:], mx[:], AF.Exp)
                            nc.vector.scalar_tensor_tensor(
                                qt_[:], qp[:], 1.0, ex[:], AluOpType.add,
                                AluOpType.max)
                        qts.append(qt_)
                    return qts

                def emit_y(j, attns):
                    if "ycompute" in abl:
                        return
                    for m in range(4):
                        for n in range(2):
                            yp = ps_y.tile([P, TC], F32, tag="yp")
                            for k in range(KC):
                                nc.tensor.matmul(
                                    yp[:], attns[k][:, m * P:(m + 1) * P],
                                    wot_t[k][:, n * TC:(n + 1) * TC],
                                    start=(k == 0), stop=(k == KC - 1))
                            ysb = p_y.tile([P, TC], F32, tag="ysb")
                            nc.vector.tensor_tensor(
                                ysb[:], yp[:], bias_bc[:, n * TC:(n + 1) * TC],
                                AluOpType.add)
                            row0 = j * TC + m * P
                            nc.sync.dma_start(
                                out[row0:row0 + P, n * TC:(n + 1) * TC], ysb[:])

                def load_qt(j):
                    ts = []
                    for c in range(KC):
                        t = p_x.tile([P, TC], BF16, tag="qtl")
                        nc.sync.dma_start(
                            t[:], qsp[c * P:(c + 1) * P, j * TC:(j + 1) * TC])
                        ts.append(t)
                    return ts

                if fuseq:
                    qtiles = {0: load_qt(0)} if do_p2 else {}
                else:
                    qtiles = {0: emit_qproj(0)} if do_p2 else {}
                attn_prev = None
                for j in range(nch if do_p2 else 0):
                    if j + 1 < nch:
                        qtiles[j + 1] = load_qt(j + 1) if fuseq else emit_qproj(j + 1)
                    qts = qtiles.pop(j)
                    if "numrbc" in abl:
                        attn_prev_new = qts
                        if attn_prev is not None:
                            emit_y(j - 1, attn_prev)
                        attn_prev = attn_prev_new
                        continue
                    # denominator for all heads: [16, TC]
                    dps_full = ps_rbc.tile([P, TC], F32, tag="rbps")
                    dps = dps_full[0:H, :]
                    for c in range(KC):
                        nc.tensor.matmul(dps[:], ksbd[c][:], qts[c][:],
                                         start=(c == 0), stop=(c == KC - 1))
                    if attn_prev is not None:
                        emit_y(j - 1, attn_prev)
                    dsb = p_dn.tile([H, TC], F32, tag="dsb")
                    nc.vector.tensor_scalar_add(dsb[:], dps[:], EPS)
                    rcp = p_dn.tile([H, TC], F32 if rbc_gps else QDT, tag="rcp")
                    with nc.allow_low_precision(reason="f32r rounding of recip"):
                        nc.vector.reciprocal(rcp[:], dsb[:])
                    attns = []
                    for p in range(NPAIR):
                        np_ = ps_num.tile([P, TC], F32, tag="nps")
                        nc.tensor.matmul(np_[:], kvbd[p][:], qts[p][:],
                                         start=True, stop=True)
                        rbs = p_rbc.tile([P, TC], F32, tag="rbs")
                        if rbc_gps:
                            nc.gpsimd.partition_broadcast(
                                rbs[0:HD, :], rcp[2 * p:2 * p + 1, :])
                            nc.gpsimd.partition_broadcast(
                                rbs[HD:P, :], rcp[2 * p + 1:2 * p + 2, :])
                        else:
                            rb = ps_rbc.tile([P, TC], F32, tag="rbps")
                            nc.tensor.matmul(rb[:], sel[p], rcp[:],
                                             start=True, stop=True)
                            nc.scalar.activation(rbs[:], rb[:], AF.Copy)
                        at = p_at.tile([P, TC], BF16 if atbf else MDT, tag="at")
                        nc.vector.tensor_tensor(at[:], np_[:], rbs[:], AluOpType.mult)
                        attns.append(at)
                    attn_prev = attns
                if do_p2:
                    emit_y(nch - 1, attn_prev)

    return nc


last_result = None


def build_compiled(ntok, n_cores=8):
    nc = bacc.Bacc("TRN2", target_bir_lowering=False, debug=False,
                   num_devices=n_cores)
    build_program(nc, ntok, qcp=True)
    nc.compile()
    from concourse.bass_interp import get_hw_module
    nc.m = get_hw_module(nc.m)
    return nc


def _run(in_maps, ntok, n_cores=8):
    # NTFF tracing is unsupported under this axon client; make sure the
    # spmd runner never takes the trace path.
    os.environ["BASS_NEVER_TRACE"] = "1"
    key = (ntok, n_cores)
    if key not in _prog_cache:
        _prog_cache[key] = build_compiled(ntok, n_cores)
    nc = _prog_cache[key]
    res = bass_utils.run_bass_kernel_spmd(nc, in_maps, list(range(n_cores)))
    global last_result
    last_result = res
    return res


def make_in_maps(x, Wq, Wk, Wv, Wo, bo, n_cores=8, mmdt="f32r", fuseq=False):
    import ml_dtypes
    if mmdt == "f32r":
        xdt = np.float32
    else:
        xdt = ml_dtypes.bfloat16
    sdt = ml_dtypes.bfloat16 if fuseq else xdt
    x = np.asarray(x, dtype=np.float32)
    B, N, _ = x.shape
    npc = B * N // n_cores  # tokens per core
    halves = N // npc       # token halves per batch item
    wqt = np.ascontiguousarray(np.asarray(Wq, np.float32).T).astype(xdt)
    wkt = np.ascontiguousarray(np.asarray(Wk, np.float32).T).astype(xdt)
    wvt = np.ascontiguousarray(np.asarray(Wv, np.float32).T).astype(xdt)
    wot = np.ascontiguousarray(np.asarray(Wo, np.float32).T).astype(xdt)
    b_ = np.asarray(bo, np.float32).reshape(1, D)
    selc = np.zeros((H, NPAIR * P), dtype=sdt)
    for p in range(NPAIR):
        selc[2 * p, p * P:p * P + HD] = 1.0
        selc[2 * p + 1, p * P + HD:(p + 1) * P] = 1.0
    dmask = np.zeros((P, P), dtype=np.float32)
    dmask[:HD, :HD] = 1.0
    dmask[HD:, HD:] = 1.0
    kmask = np.zeros((P, KC * H), dtype=np.float32)
    for c in range(KC):
        kmask[:HD, c * H + 2 * c] = 1.0
        kmask[HD:, c * H + 2 * c + 1] = 1.0
    in_maps = []
    for i in range(n_cores):
        b, h = divmod(i, halves)
        xs = np.ascontiguousarray(x[b, h * npc:(h + 1) * npc, :].T).astype(xdt)
        key = "selcb" if fuseq else "selc"
        in_maps.append({"xt": xs, "wqt": wqt, "wkt": wkt, "wvt": wvt,
                        "wot": wot, "bias": b_, key: selc,
                        "dmask": dmask, "kmask": kmask})
    return in_maps, npc


_prog_cache = {}


def kernel(x, Wq, Wk, Wv, Wo, bo):
    x = np.asarray(x, dtype=np.float32)
    B, N, _ = x.shape
    n_cores = 8
    in_maps, npc = make_in_maps(x, Wq, Wk, Wv, Wo, bo, n_cores)
    halves = N // npc
    res = _run(in_maps, npc, n_cores)
    out = np.empty((B, N, D), dtype=np.float32)
    for i in range(n_cores):
        b, h = divmod(i, halves)
        out[b, h * npc:(h + 1) * npc, :] = res.results[i]["out"]
    return out



# revision 9
# speedup vs baseline: 1.0608x; 1.0608x over previous
"""Bidirectional linear attention kernel for 8 TRN2 NeuronCores.

Sharding: core i handles batch b = i//2, token half i%2 (4096 tokens each).
Per-head KV aggregation is completed with an AllReduce of the packed
[kv | ksum] accumulator (528KB) within core pairs {0,1},{2,3},{4,5},{6,7}.

Layout strategy (per core):
  - x arrives host-transposed as xT [D, Ntok] (feature-major).
  - Phase 1: k, v computed token-major ([tok, D]) with fp32r matmuls;
    phi(k) = max(k+1, exp(min(k,0))) fused ACT+DVE; kv per head-pair via
    bf16 matmuls with a ones-column appended to v so ksum falls out of the
    same matmul (out [128, 129] per pair), accumulated into SBUF.
  - AllReduce pairs over the packed [128, 1032] kv/ksum buffer.
  - Phase 2: qT computed feature-major; numerator^T = blockdiag(kv)^T @ qT;
    denominator for all heads via blockdiag-ksum matmul -> [16, T];
    reciprocal broadcast back to 64 partitions/head with a small select
    matmul; divide; output projection token-major + bias.
"""

import os
import sys

import numpy as np

for _p in ("/opt/trn_rl_repo", "/root/.axon_site/_ro/trn_rl_repo"):
    if os.path.isdir(_p) and _p not in sys.path:
        sys.path.append(_p)

from contextlib import ExitStack

import concourse.bacc as bacc
import concourse.tile as tile
from concourse import mybir
from concourse.alu_op_type import AluOpType
from concourse import bass_utils

F32 = mybir.dt.float32
F32R = mybir.dt.float32r
BF16 = mybir.dt.bfloat16
AF = mybir.ActivationFunctionType

D = 1024        # model dim
H = 16          # heads
HD = 64         # head dim
P = 128         # partitions
NPAIR = 8       # head pairs
KC = 8          # feature chunks of 128
TC = 512        # token chunk
EPS = 1e-6
PW = 129        # pair width in the packed kv|ksum layout


def r(ap):
    return ap.bitcast(F32R)


def build_program(nc, ntok, use_cc=True, reps=1, mmdt="f32r",
                  do_p1=True, do_p2=True, abl=(), fuseq=False, rbc_gps=False,
                  atbf=False, qcp=False, kcp=False):
    abl = set(abl)
    nch = ntok // TC
    MDT = F32R if mmdt == "f32r" else BF16
    XDT = F32 if mmdt == "f32r" else BF16

    def rr(ap):
        return ap.bitcast(F32R) if mmdt == "f32r" else ap

    xt = nc.dram_tensor("xt", [D, ntok], XDT, kind="ExternalInput").ap()
    wqt = nc.dram_tensor("wqt", [D, D], XDT, kind="ExternalInput").ap()
    wkt = nc.dram_tensor("wkt", [D, D], XDT, kind="ExternalInput").ap()
    wvt = nc.dram_tensor("wvt", [D, D], XDT, kind="ExternalInput").ap()
    wot = (nc.dram_tensor("wotb", [D, D], BF16, kind="ExternalInput").ap()
           if atbf else
           nc.dram_tensor("wot", [D, D], XDT, kind="ExternalInput").ap())
    bias = nc.dram_tensor("bias", [1, D], F32, kind="ExternalInput").ap()
    out = nc.dram_tensor("out", [ntok, D], F32, kind="ExternalOutput").ap()

    cc_in = nc.dram_tensor("cc_in", [P, NPAIR * PW], F32).ap()
    cc_out = nc.dram_tensor("cc_out", [P, NPAIR * PW], F32).ap()
    QDT = BF16 if fuseq else MDT
    qsp = nc.dram_tensor("qsp", [D, ntok], BF16).ap() if fuseq else None

    with tile.TileContext(nc) as tc, ExitStack() as es:
        p_w = es.enter_context(tc.tile_pool(name="w", bufs=24 if fuseq else 16))
        p_x = es.enter_context(tc.tile_pool(name="x", bufs=9))
        p_scr = es.enter_context(tc.tile_pool(name="scr", bufs=3 if atbf else 2))
        p_qc = es.enter_context(tc.tile_pool(name="qc", bufs=3))
        p_c1 = es.enter_context(tc.tile_pool(name="c1", bufs=1))
        p_c8 = es.enter_context(tc.tile_pool(name="c8", bufs=8))

        # bias broadcast [1, D] -> [128, D]
        b_row = p_c1.tile([1, D], F32, tag="brow")
        nc.sync.dma_start(b_row[:], bias[:, :])
        bias_bc = p_c1.tile([P, D], F32, tag="bias")
        nc.gpsimd.partition_broadcast(bias_bc[:], b_row[:])

        # select matrices for the reciprocal broadcast (host-prepared constant)
        selc = nc.dram_tensor("selcb" if fuseq else "selc", [H, NPAIR * P],
                          BF16 if fuseq else XDT, kind="ExternalInput").ap()
        sel_t = p_c1.tile([H, NPAIR * P], QDT, tag="sel")
        nc.sync.dma_start(sel_t[:], selc[:, :] if fuseq else rr(selc[:, :]))
        sel = [sel_t[:, p * P:(p + 1) * P] for p in range(NPAIR)]

        # masks for block-diag kv / ksum lhsT construction
        dmask = nc.dram_tensor("dmask", [P, P], F32, kind="ExternalInput").ap()
        dmask_t = p_c1.tile([P, P], F32, tag="dmask")
        nc.sync.dma_start(dmask_t[:], dmask[:, :])
        kmask = nc.dram_tensor("kmask", [P, KC * H], F32,
                               kind="ExternalInput").ap()
        kmask_t = p_c1.tile([P, KC * H], F32, tag="kmask")
        nc.sync.dma_start(kmask_t[:], kmask[:, :])

        # packed kv | ksum accumulator
        kvks = p_c1.tile([P, NPAIR * PW], F32, tag="kvks")
        kvks3 = kvks.rearrange("p (a b) -> p a b", b=PW)

        def load_xt(j):
            ts = []
            for k in range(KC):
                t = p_x.tile([P, TC], MDT, tag="xt")
                nc.sync.dma_start(
                    t[:], rr(xt[k * P:(k + 1) * P, j * TC:(j + 1) * TC]))
                ts.append(t)
            return ts

        for _rep in range(reps):
            # ---- weights (phase 1) ----
            wkt_t = []
            wvt_t = []
            wq1_t = []
            if do_p1:
                for k in range(KC):
                    t = p_w.tile([P, D], MDT, tag="w")
                    nc.sync.dma_start(t[:], rr(wkt[k * P:(k + 1) * P, :]))
                    wkt_t.append(t)
                for k in range(KC):
                    t = p_w.tile([P, D], MDT, tag="w")
                    nc.sync.dma_start(t[:], rr(wvt[k * P:(k + 1) * P, :]))
                    wvt_t.append(t)
                if fuseq:
                    for k in range(KC):
                        t = p_w.tile([P, D], MDT, tag="w")
                        nc.sync.dma_start(t[:], rr(wqt[k * P:(k + 1) * P, :]))
                        wq1_t.append(t)
            nc.vector.memset(kvks[:], 0.0)

            # ================= phase 1: k, v, kv, ksum =================
            with ExitStack() as es1:
                ps_k = es1.enter_context(tc.tile_pool(name="psk", bufs=2, space="PSUM"))
                ps_v = es1.enter_context(tc.tile_pool(name="psv", bufs=2, space="PSUM"))
                ps_kv = es1.enter_context(tc.tile_pool(name="pskv", bufs=2, space="PSUM"))
                p_kphi = es1.enter_context(tc.tile_pool(name="kphi", bufs=6))
                p_v = es1.enter_context(tc.tile_pool(name="vsb", bufs=6))
                if fuseq:
                    ps_q1 = es1.enter_context(
                        tc.tile_pool(name="psq1", bufs=2, space="PSUM"))
                    p_q1 = es1.enter_context(tc.tile_pool(name="q1", bufs=3))

                for j in range(nch if do_p1 else 0):
                    xts = load_xt(j)
                    kphis = []
                    vsbs = []
                    for m in range(4):
                        xm = [xts[k][:, m * P:(m + 1) * P] for k in range(KC)]
                        # k projection + phi -> bf16 [128, 1024]
                        kph = p_kphi.tile([P, D], BF16, tag="kphi")
                        for n in range(2):
                            kp = ps_k.tile([P, TC], F32, tag="kp")
                            for k in range(KC):
                                nc.tensor.matmul(
                                    kp[:], xm[k], wkt_t[k][:, n * TC:(n + 1) * TC],
                                    start=(k == 0), stop=(k == KC - 1))
                            if "phi" in abl:
                                nc.scalar.activation(
                                    kph[:, n * TC:(n + 1) * TC], kp[:], AF.Copy)
                            elif kcp:
                                kc_ = p_qc.tile([P, TC], F32, tag="kc")
                                nc.scalar.activation(kc_[:], kp[:], AF.Copy)
                                mx = p_scr.tile([P, TC], F32, tag="mx")
                                nc.vector.tensor_scalar_min(mx[:], kc_[:], 0.0)
                                ex = p_scr.tile([P, TC], F32, tag="ex")
                                nc.scalar.activation(ex[:], mx[:], AF.Exp)
                                nc.vector.scalar_tensor_tensor(
                                    kph[:, n * TC:(n + 1) * TC], kc_[:], 1.0,
                                    ex[:], AluOpType.add, AluOpType.max)
                            else:
                                mx = p_scr.tile([P, TC], F32, tag="mx")
                                nc.vector.tensor_scalar_min(mx[:], kp[:], 0.0)
                                ex = p_scr.tile([P, TC], F32, tag="ex")
                                nc.scalar.activation(ex[:], mx[:], AF.Exp)
                                nc.vector.scalar_tensor_tensor(
                                    kph[:, n * TC:(n + 1) * TC], kp[:], 1.0, ex[:],
                                    AluOpType.add, AluOpType.max)
                        kphis.append(kph)
                        # v projection -> bf16 [128, 1032] interleaved with ones cols
                        vsb = p_v.tile([P, NPAIR * PW], BF16, tag="vsb")
                        vsb3 = vsb.rearrange("p (a b) -> p a b", b=PW)
                        for n in range(2):
                            vp = ps_v.tile([P, TC], F32, tag="vp")
                            for k in range(KC):
                                nc.tensor.matmul(
                                    vp[:], xm[k], wvt_t[k][:, n * TC:(n + 1) * TC],
                                    start=(k == 0), stop=(k == KC - 1))
                            if "vcopy" not in abl:
                                nc.scalar.activation(
                                    vsb3[:, 4 * n:4 * n + 4, 0:P],
                                    vp.rearrange("p (a b) -> p a b", b=P), AF.Copy)
                        if "vcopy" not in abl:
                            nc.vector.memset(vsb3[:, :, P:PW], 1.0)
                        else:
                            nc.vector.memset(vsb[:], 1.0)
                        vsbs.append(vsb)
                    # fused q projection + phi + spill to DRAM (bf16)
                    if fuseq:
                        for c in range(KC):
                            qp1 = ps_q1.tile([P, TC], F32, tag="qp1")
                            for k in range(KC):
                                nc.tensor.matmul(
                                    qp1[:], wq1_t[k][:, c * P:(c + 1) * P],
                                    xts[k][:],
                                    start=(k == 0), stop=(k == KC - 1))
                            q1 = p_q1.tile([P, TC], BF16, tag="q1")
                            if "phi" in abl:
                                nc.scalar.activation(q1[:], qp1[:], AF.Copy)
                            else:
                                mx = p_scr.tile([P, TC], F32, tag="mx")
                                nc.vector.tensor_scalar_min(mx[:], qp1[:], 0.0)
                                ex = p_scr.tile([P, TC], F32, tag="ex")
                                nc.scalar.activation(ex[:], mx[:], AF.Exp)
                                nc.vector.scalar_tensor_tensor(
                                    q1[:], qp1[:], 1.0, ex[:], AluOpType.add,
                                    AluOpType.max)
                            nc.sync.dma_start(
                                qsp[c * P:(c + 1) * P, j * TC:(j + 1) * TC],
                                q1[:])
                    # kv + ksum accumulation per pair
                    for p in range(NPAIR if "kv" not in abl else 0):
                        kvp = ps_kv.tile([P, PW], F32, tag="kvp")
                        for m in range(4):
                            nc.tensor.matmul(
                                kvp[:], kphis[m][:, p * P:(p + 1) * P],
                                vsbs[m].rearrange("p (a b) -> p a b", b=PW)[:, p, :],
                                start=(m == 0), stop=(m == 3))
                        nc.vector.tensor_tensor(
                            kvks3[:, p, :], kvp[:], kvks3[:, p, :], AluOpType.add)

            # ================= allreduce within pairs =================
            nc.sync.dma_start(cc_in[:, :], kvks[:])
            if use_cc:
                nc.gpsimd.collective_compute(
                    "AllReduce", AluOpType.add,
                    replica_groups=[[0, 1], [2, 3], [4, 5], [6, 7]],
                    ins=[cc_in[:, :]], outs=[cc_out[:, :]])
            else:
                nc.sync.dma_start(cc_out[:, :], cc_in[:, :])
            red = p_c1.tile([P, NPAIR * PW], F32, tag="red")
            nc.sync.dma_start(red[:], cc_out[:, :])
            red3 = red.rearrange("p (a b) -> p a b", b=PW)

            # block-diag kv lhsT per pair (diag-mask multiply), block ksum lhsT
            # per chunk (per-partition scalar multiply against a column mask).
            kvbd = []
            ksbd = []
            for p in range(NPAIR):
                t = p_c8.tile([P, P], QDT, tag="kvbd")
                nc.vector.tensor_tensor(t[:], red3[:, p, 0:P], dmask_t[:],
                                        AluOpType.mult)
                kvbd.append(t)
            for c in range(KC):
                t = p_c8.tile([P, H], QDT, tag="ksbd")
                nc.vector.tensor_scalar(
                    t[:], kmask_t[:, c * H:(c + 1) * H], red3[:, c, P:PW], None,
                    AluOpType.mult)
                ksbd.append(t)

            # ---- weights (phase 2) ----
            wqt_t = []
            wot_t = []
            if do_p2:
                for k in range(KC):
                    t = p_w.tile([P, D], MDT, tag="w")
                    nc.sync.dma_start(t[:], rr(wqt[k * P:(k + 1) * P, :]))
                    wqt_t.append(t)
                for k in range(KC):
                    if atbf:
                        t = p_w.tile([P, D], BF16, tag="w")
                        nc.sync.dma_start(t[:], wot[k * P:(k + 1) * P, :])
                    else:
                        t = p_w.tile([P, D], MDT, tag="w")
                        nc.sync.dma_start(t[:], rr(wot[k * P:(k + 1) * P, :]))
                    wot_t.append(t)

            # ================= phase 2: q, numerator, denom, y =================
            with ExitStack() as es2:
                ps_q = es2.enter_context(tc.tile_pool(name="psq", bufs=2, space="PSUM"))
                ps_num = es2.enter_context(tc.tile_pool(name="psnum", bufs=2, space="PSUM"))
                ps_rbc = es2.enter_context(tc.tile_pool(name="psrbc", bufs=2, space="PSUM"))
                ps_y = es2.enter_context(tc.tile_pool(name="psy", bufs=2, space="PSUM"))
                p_qt = es2.enter_context(tc.tile_pool(name="qt", bufs=16))
                p_at = es2.enter_context(tc.tile_pool(name="at", bufs=16 if atbf else 11))
                p_rbc = es2.enter_context(tc.tile_pool(name="rbc", bufs=3))
                p_y = es2.enter_context(tc.tile_pool(name="ysb", bufs=3))
                p_dn = es2.enter_context(tc.tile_pool(name="dn", bufs=1))

                def emit_qproj(j):
                    xts = load_xt(j)
                    qts = []
                    for c in range(KC):
                        qp = ps_q.tile([P, TC], F32, tag="qp")
                        for k in range(KC):
                            nc.tensor.matmul(
                                qp[:], wqt_t[k][:, c * P:(c + 1) * P], xts[k][:],
                                start=(k == 0), stop=(k == KC - 1))
                        qt_ = p_qt.tile([P, TC], MDT, tag="qt")
                        if "phi" in abl:
                            nc.scalar.activation(qt_[:], qp[:], AF.Copy)
                        elif qcp:
                            # release the PSUM tile with one ACT copy, then
                            # run the phi chain from SBUF so the PE is not
                            # gated on the full DVE/ACT consumer chain
                            qc = p_qc.tile([P, TC], F32, tag="qc")
                            nc.scalar.activation(qc[:], qp[:], AF.Copy)
                            mx = p_scr.tile([P, TC], F32, tag="mx")
                            nc.vector.tensor_scalar_min(mx[:], qc[:], 0.0)
                            ex = p_scr.tile([P, TC], F32, tag="ex")
                            nc.scalar.activation(ex[:], mx[:], AF.Exp)
                            nc.vector.scalar_tensor_tensor(
                                qt_[:], qc[:], 1.0, ex[:], AluOpType.add,
                                AluOpType.max)
                        else:
                            mx = p_scr.tile([P, TC], F32, tag="mx")
                            nc.vector.tensor_scalar_min(mx[:], qp[:], 0.0)
                            ex = p_scr.tile([P, TC], F32, tag="ex")
                            nc.scalar.activation(ex[:], mx[:], AF.Exp)
                            nc.vector.scalar_tensor_tensor(
                                qt_[:], qp[:], 1.0, ex[:], AluOpType.add,
                                AluOpType.max)
                        qts.append(qt_)
                    return qts

                def emit_y(j, attns):
                    if "ycompute" in abl:
                        return
                    for m in range(4):
                        for n in range(2):
                            yp = ps_y.tile([P, TC], F32, tag="yp")
                            for k in range(KC):
                                nc.tensor.matmul(
                                    yp[:], attns[k][:, m * P:(m + 1) * P],
                                    wot_t[k][:, n * TC:(n + 1) * TC],
                                    start=(k == 0), stop=(k == KC - 1))
                            ysb = p_y.tile([P, TC], F32, tag="ysb")
                            nc.vector.tensor_tensor(
                                ysb[:], yp[:], bias_bc[:, n * TC:(n + 1) * TC],
                                AluOpType.add)
                            row0 = j * TC + m * P
                            nc.sync.dma_start(
                                out[row0:row0 + P, n * TC:(n + 1) * TC], ysb[:])

                def load_qt(j):
                    ts = []
                    for c in range(KC):
                        t = p_x.tile([P, TC], BF16, tag="qtl")
                        nc.sync.dma_start(
                            t[:], qsp[c * P:(c + 1) * P, j * TC:(j + 1) * TC])
                        ts.append(t)
                    return ts

                if fuseq:
                    qtiles = {0: load_qt(0)} if do_p2 else {}
                else:
                    qtiles = {0: emit_qproj(0)} if do_p2 else {}
                attn_prev = None
                for j in range(nch if do_p2 else 0):
                    if j + 1 < nch:
                        qtiles[j + 1] = load_qt(j + 1) if fuseq else emit_qproj(j + 1)
                    qts = qtiles.pop(j)
                    if "numrbc" in abl:
                        attn_prev_new = qts
                        if attn_prev is not None:
                            emit_y(j - 1, attn_prev)
                        attn_prev = attn_prev_new
                        continue
                    # denominator for all heads: [16, TC]
                    dps_full = ps_rbc.tile([P, TC], F32, tag="rbps")
                    dps = dps_full[0:H, :]
                    for c in range(KC):
                        nc.tensor.matmul(dps[:], ksbd[c][:], qts[c][:],
                                         start=(c == 0), stop=(c == KC - 1))
                    if attn_prev is not None:
                        emit_y(j - 1, attn_prev)
                    dsb = p_dn.tile([H, TC], F32, tag="dsb")
                    nc.vector.tensor_scalar_add(dsb[:], dps[:], EPS)
                    rcp = p_dn.tile([H, TC], F32 if rbc_gps else QDT, tag="rcp")
                    with nc.allow_low_precision(reason="f32r rounding of recip"):
                        nc.vector.reciprocal(rcp[:], dsb[:])
                    attns = []
                    for p in range(NPAIR):
                        np_ = ps_num.tile([P, TC], F32, tag="nps")
                        nc.tensor.matmul(np_[:], kvbd[p][:], qts[p][:],
                                         start=True, stop=True)
                        rbs = p_rbc.tile([P, TC], F32, tag="rbs")
                        if rbc_gps:
                            nc.gpsimd.partition_broadcast(
                                rbs[0:HD, :], rcp[2 * p:2 * p + 1, :])
                            nc.gpsimd.partition_broadcast(
                                rbs[HD:P, :], rcp[2 * p + 1:2 * p + 2, :])
                        else:
                            rb = ps_rbc.tile([P, TC], F32, tag="rbps")
                            nc.tensor.matmul(rb[:], sel[p], rcp[:],
                                             start=True, stop=True)
                            nc.scalar.activation(rbs[:], rb[:], AF.Copy)
                        at = p_at.tile([P, TC], BF16 if atbf else MDT, tag="at")
                        nc.vector.tensor_tensor(at[:], np_[:], rbs[:], AluOpType.mult)
                        attns.append(at)
                    attn_prev = attns
                if do_p2:
                    emit_y(nch - 1, attn_prev)

    return nc


def build_program2(nc, ntok, reps=1, use_cc=True, do_p1=True, do_p2=True,
                   abl=(), qpre=3):
    """v2: all-bf16 matmuls; numerator folded into the output projection via
    M = blockdiag(kv) @ Wo^T; reciprocal applied to q via Pool broadcast;
    all four weight matrices prefetched at rep start (bf16 halves SBUF and
    DMA so they fit resident simultaneously); q-projection runs `qpre`
    chunks ahead so the pair AllReduce hides behind PE work.
    """
    abl = set(abl)
    nch = ntok // TC

    xt = nc.dram_tensor("xt", [D, ntok], BF16, kind="ExternalInput").ap()
    wkb = nc.dram_tensor("wkb", [D, D], BF16, kind="ExternalInput").ap()
    wvb = nc.dram_tensor("wvb", [D, D], BF16, kind="ExternalInput").ap()
    wqb = nc.dram_tensor("wqb", [D, D], BF16, kind="ExternalInput").ap()
    wob = nc.dram_tensor("wob", [D, D], BF16, kind="ExternalInput").ap()
    bias = nc.dram_tensor("bias", [1, D], F32, kind="ExternalInput").ap()
    ident = nc.dram_tensor("ident", [P, P], BF16, kind="ExternalInput").ap()
    dmask = nc.dram_tensor("dmask", [P, P], F32, kind="ExternalInput").ap()
    kmask = nc.dram_tensor("kmask", [P, KC * H], F32,
                           kind="ExternalInput").ap()
    out = nc.dram_tensor("out", [ntok, D], F32, kind="ExternalOutput").ap()

    cc_in = nc.dram_tensor("cc_in", [P, NPAIR * PW], F32).ap()
    cc_out = nc.dram_tensor("cc_out", [P, NPAIR * PW], F32).ap()

    with tile.TileContext(nc) as tc, ExitStack() as es:
        p_w = es.enter_context(tc.tile_pool(name="w", bufs=32))
        p_x = es.enter_context(tc.tile_pool(name="x", bufs=9))
        p_scr = es.enter_context(tc.tile_pool(name="scr", bufs=2))
        p_qc = es.enter_context(tc.tile_pool(name="qc", bufs=3))
        p_qt = es.enter_context(tc.tile_pool(name="qt", bufs=32))
        p_kphi = es.enter_context(tc.tile_pool(name="kphi", bufs=6))
        p_v = es.enter_context(tc.tile_pool(name="vsb", bufs=6))
        p_m = es.enter_context(tc.tile_pool(name="m", bufs=8))
        p_kv8 = es.enter_context(tc.tile_pool(name="kv8", bufs=8))
        p_rbs = es.enter_context(tc.tile_pool(name="rbs", bufs=6))
        p_dn = es.enter_context(tc.tile_pool(name="dn", bufs=2))
        p_rfl = es.enter_context(tc.tile_pool(name="rfl", bufs=1))
        p_y = es.enter_context(tc.tile_pool(name="ysb", bufs=3))
        p_c1 = es.enter_context(tc.tile_pool(name="c1", bufs=1))

        # constants
        b_row = p_c1.tile([1, D], F32, tag="brow")
        nc.sync.dma_start(b_row[:], bias[:, :])
        bias_bc = p_c1.tile([P, D], F32, tag="bias")
        nc.gpsimd.partition_broadcast(bias_bc[:], b_row[:])
        ident_t = p_c1.tile([P, P], BF16, tag="ident")
        nc.sync.dma_start(ident_t[:], ident[:, :])
        dmask_t = p_c1.tile([P, P], F32, tag="dmask")
        nc.sync.dma_start(dmask_t[:], dmask[:, :])
        kmask_t = p_c1.tile([P, KC * H], F32, tag="kmask")
        nc.sync.dma_start(kmask_t[:], kmask[:, :])

        kvks = p_c1.tile([P, NPAIR * PW], F32, tag="kvks")
        kvks3 = kvks.rearrange("p (a b) -> p a b", b=PW)

        def load_w(src):
            ts = []
            for k in range(KC):
                t = p_w.tile([P, D], BF16, tag="w")
                nc.sync.dma_start(t[:], src[k * P:(k + 1) * P, :])
                ts.append(t)
            return ts

        def load_xt(j):
            ts = []
            for k in range(KC):
                t = p_x.tile([P, TC], BF16, tag="xt")
                nc.sync.dma_start(
                    t[:], xt[k * P:(k + 1) * P, j * TC:(j + 1) * TC])
                ts.append(t)
            return ts

        def phi_from(psum, dst, via):
            # dst = max(src + 1, exp(min(src, 0))), src read once into `via`
            nc.scalar.activation(via[:], psum[:], AF.Copy)
            mx = p_scr.tile([P, TC], BF16, tag="mx")
            nc.vector.tensor_scalar_min(mx[:], via[:], 0.0)
            ex = p_scr.tile([P, TC], BF16, tag="ex")
            nc.scalar.activation(ex[:], mx[:], AF.Exp)
            nc.vector.scalar_tensor_tensor(
                dst, via[:], 1.0, ex[:], AluOpType.add, AluOpType.max)

        for _rep in range(reps):
            # interleave wk with x(0) so the first k-proj group is DMA-paced
            # rather than blocked on the full weight prefetch
            wkt = []
            xts0 = []
            if do_p1:
                for k in range(KC):
                    t = p_w.tile([P, D], BF16, tag="w")
                    nc.sync.dma_start(t[:], wkb[k * P:(k + 1) * P, :])
                    wkt.append(t)
                    tx = p_x.tile([P, TC], BF16, tag="xt")
                    nc.sync.dma_start(tx[:], xt[k * P:(k + 1) * P, 0:TC])
                    xts0.append(tx)
            wvt = load_w(wvb) if do_p1 else []
            wqt = load_w(wqb) if do_p2 else []
            wot = load_w(wob) if do_p2 else []
            nc.vector.memset(kvks[:], 0.0)

            # ================= phase 1: k, v, kv|ksum =================
            with ExitStack() as es1:
                ps_k = es1.enter_context(
                    tc.tile_pool(name="psk", bufs=2, space="PSUM"))
                ps_v = es1.enter_context(
                    tc.tile_pool(name="psv", bufs=2, space="PSUM"))
                ps_kv = es1.enter_context(
                    tc.tile_pool(name="pskv", bufs=2, space="PSUM"))

                for j in range(nch if do_p1 else 0):
                    xts = xts0 if j == 0 else load_xt(j)
                    kphis = []
                    vsbs = []
                    for m in range(4):
                        xm = [xts[k][:, m * P:(m + 1) * P] for k in range(KC)]
                        kph = p_kphi.tile([P, D], BF16, tag="kphi")
                        for n in range(2):
                            kp = ps_k.tile([P, TC], F32, tag="kp")
                            for k in range(KC):
                                nc.tensor.matmul(
                                    kp[:], xm[k],
                                    wkt[k][:, n * TC:(n + 1) * TC],
                                    start=(k == 0), stop=(k == KC - 1))
                            if "phi" in abl:
                                nc.scalar.activation(
                                    kph[:, n * TC:(n + 1) * TC], kp[:],
                                    AF.Copy)
                            else:
                                kc_ = p_qc.tile([P, TC], BF16, tag="qc")
                                phi_from(kp, kph[:, n * TC:(n + 1) * TC], kc_)
                        kphis.append(kph)
                        vsb = p_v.tile([P, NPAIR * PW], BF16, tag="vsb")
                        vsb3 = vsb.rearrange("p (a b) -> p a b", b=PW)
                        for n in range(2):
                            vp = ps_v.tile([P, TC], F32, tag="vp")
                            for k in range(KC):
                                nc.tensor.matmul(
                                    vp[:], xm[k],
                                    wvt[k][:, n * TC:(n + 1) * TC],
                                    start=(k == 0), stop=(k == KC - 1))
                            nc.scalar.activation(
                                vsb3[:, 4 * n:4 * n + 4, 0:P],
                                vp.rearrange("p (a b) -> p a b", b=P),
                                AF.Copy)
                        nc.vector.memset(vsb3[:, :, P:PW], 1.0)
                        vsbs.append(vsb)
                    for p in range(NPAIR if "kv" not in abl else 0):
                        kvp = ps_kv.tile([P, PW], F32, tag="kvp")
                        for m in range(4):
                            nc.tensor.matmul(
                                kvp[:], kphis[m][:, p * P:(p + 1) * P],
                                vsbs[m].rearrange(
                                    "p (a b) -> p a b", b=PW)[:, p, :],
                                start=(m == 0), stop=(m == 3))
                        nc.vector.tensor_tensor(
                            kvks3[:, p, :], kvp[:], kvks3[:, p, :],
                            AluOpType.add)

            # ================= allreduce within pairs =================
            nc.sync.dma_start(cc_in[:, :], kvks[:])
            if use_cc:
                nc.gpsimd.collective_compute(
                    "AllReduce", AluOpType.add,
                    replica_groups=[[0, 1], [2, 3], [4, 5], [6, 7]],
                    ins=[cc_in[:, :]], outs=[cc_out[:, :]])
            else:
                nc.sync.dma_start(cc_out[:, :], cc_in[:, :])
            red = p_c1.tile([P, NPAIR * PW], F32, tag="red")
            nc.sync.dma_start(red[:], cc_out[:, :])
            red3 = red.rearrange("p (a b) -> p a b", b=PW)

            # ================= phase 2: q, den, y = q' @ M =================
            with ExitStack() as es2:
                ps_q = es2.enter_context(
                    tc.tile_pool(name="psq", bufs=2, space="PSUM"))
                ps_dn = es2.enter_context(
                    tc.tile_pool(name="psdn", bufs=2, space="PSUM"))
                ps_y = es2.enter_context(
                    tc.tile_pool(name="psy", bufs=2, space="PSUM"))
                ps_mt = es2.enter_context(
                    tc.tile_pool(name="psmt", bufs=1, space="PSUM"))

                def emit_qproj(j):
                    xts = load_xt(j)
                    qts = []
                    for c in range(KC):
                        qp = ps_q.tile([P, TC], F32, tag="qp")
                        for k in range(KC):
                            nc.tensor.matmul(
                                qp[:], wqt[k][:, c * P:(c + 1) * P], xts[k][:],
                                start=(k == 0), stop=(k == KC - 1))
                        qt_ = p_qt.tile([P, TC], BF16, tag="qt")
                        if "phi" in abl:
                            nc.scalar.activation(qt_[:], qp[:], AF.Copy)
                        else:
                            qc = p_qc.tile([P, TC], BF16, tag="qc")
                            phi_from(qp, qt_[:], qc)
                        qts.append(qt_)
                    return qts

                def emit_den(j, qts):
                    # denominator for all heads -> reciprocal on partition 0
                    dps_full = ps_dn.tile([P, TC], F32, tag="dn")
                    dps = dps_full[0:H, :]
                    for c in range(KC):
                        nc.tensor.matmul(dps[:], ksbd[c][:], qts[c][:],
                                         start=(c == 0), stop=(c == KC - 1))
                    dsb = p_dn.tile([H, TC], F32, tag="dsb")
                    nc.vector.tensor_scalar_add(dsb[:], dps[:], EPS)
                    rcp = p_dn.tile([H, TC], BF16, tag="rcp")
                    with nc.allow_low_precision(reason="bf16 recip"):
                        nc.vector.reciprocal(rcp[:], dsb[:])
                    rfl = p_rfl.tile([1, H * TC], BF16, tag="rfl")
                    rfl3 = rfl.rearrange("p (a b) -> p a b", b=TC)
                    nc.sync.dma_start(rfl3[:, :, :], rcp[:, :])
                    if "scale" in abl:
                        return
                    for c in range(KC):
                        rbe = p_rbs.tile([P, TC], BF16, tag="rbs")
                        nc.gpsimd.partition_broadcast(
                            rbe[:, :], rfl3[:, 2 * c, :])
                        rbo = p_rbs.tile([P, TC], BF16, tag="rbs")
                        nc.gpsimd.partition_broadcast(
                            rbo[:, :], rfl3[:, 2 * c + 1, :])
                        nc.vector.tensor_tensor(
                            qts[c][0:HD, :], qts[c][0:HD, :], rbe[0:HD, :],
                            AluOpType.mult)
                        nc.vector.tensor_tensor(
                            qts[c][HD:P, :], qts[c][HD:P, :], rbo[HD:P, :],
                            AluOpType.mult)

                def emit_y(j, qts):
                    if "y" in abl:
                        return
                    for m in range(4):
                        for n in range(2):
                            yp = ps_y.tile([P, TC], F32, tag="yp")
                            for c in range(KC):
                                nc.tensor.matmul(
                                    yp[:], qts[c][:, m * P:(m + 1) * P],
                                    m_t[c][:, n * TC:(n + 1) * TC],
                                    start=(c == 0), stop=(c == KC - 1))
                            ysb = p_y.tile([P, TC], F32, tag="ysb")
                            nc.vector.tensor_tensor(
                                ysb[:], yp[:],
                                bias_bc[:, n * TC:(n + 1) * TC],
                                AluOpType.add)
                            row0 = j * TC + m * P
                            nc.sync.dma_start(
                                out[row0:row0 + P, n * TC:(n + 1) * TC],
                                ysb[:])

                if do_p2:
                    qtiles = {}
                    for j in range(min(qpre, nch)):
                        qtiles[j] = emit_qproj(j)

                    # M = blockdiag(kv) @ Wo^T  (per pair: transpose + 2 mm)
                    m_t = []
                    ksbd = []
                    for p in range(NPAIR):
                        kvbd = p_kv8.tile([P, P], BF16, tag="kvbd")
                        nc.vector.tensor_tensor(
                            kvbd[:], red3[:, p, 0:P], dmask_t[:],
                            AluOpType.mult)
                        tp = ps_mt.tile([P, P], BF16, tag="tp")
                        nc.tensor.transpose(tp[:], kvbd[:], ident_t[:])
                        kvt = p_kv8.tile([P, P], BF16, tag="kvt")
                        nc.scalar.activation(kvt[:], tp[:], AF.Copy)
                        mt = p_m.tile([P, D], BF16, tag="m")
                        for n in range(2):
                            mm = ps_mt.tile([P, TC], F32, tag="mm")
                            nc.tensor.matmul(mm[:], kvt[:],
                                             wot[p][:, n * TC:(n + 1) * TC],
                                             start=True, stop=True)
                            nc.scalar.activation(
                                mt[:, n * TC:(n + 1) * TC], mm[:], AF.Copy)
                        m_t.append(mt)
                        t = p_kv8.tile([P, H], BF16, tag="ksbd")
                        nc.vector.tensor_scalar(
                            t[:], kmask_t[:, p * H:(p + 1) * H],
                            red3[:, p, P:PW], None, AluOpType.mult)
                        ksbd.append(t)

                    emit_den(0, qtiles[0])
                    for j in range(nch):
                        if j + 1 < nch:
                            emit_den(j + 1, qtiles[j + 1])
                        if j + qpre < nch:
                            qtiles[j + qpre] = emit_qproj(j + qpre)
                        emit_y(j, qtiles.pop(j))

    return nc


def make_in_maps2(x, Wq, Wk, Wv, Wo, bo, n_cores=8):
    import ml_dtypes
    bf = ml_dtypes.bfloat16
    x = np.asarray(x, dtype=np.float32)
    B, N, _ = x.shape
    npc = B * N // n_cores
    halves = N // npc
    wkb = np.ascontiguousarray(np.asarray(Wk, np.float32).T).astype(bf)
    wvb = np.ascontiguousarray(np.asarray(Wv, np.float32).T).astype(bf)
    wqb = np.ascontiguousarray(np.asarray(Wq, np.float32).T).astype(bf)
    wob = np.ascontiguousarray(np.asarray(Wo, np.float32).T).astype(bf)
    b_ = np.asarray(bo, np.float32).reshape(1, D)
    ident = np.eye(P, dtype=bf)
    dmask = np.zeros((P, P), dtype=np.float32)
    dmask[:HD, :HD] = 1.0
    dmask[HD:, HD:] = 1.0
    kmask = np.zeros((P, KC * H), dtype=np.float32)
    for c in range(KC):
        kmask[:HD, c * H + 2 * c] = 1.0
        kmask[HD:, c * H + 2 * c + 1] = 1.0
    in_maps = []
    for i in range(n_cores):
        b, h = divmod(i, halves)
        xs = np.ascontiguousarray(x[b, h * npc:(h + 1) * npc, :].T).astype(bf)
        in_maps.append({"xt": xs, "wkb": wkb, "wvb": wvb, "wqb": wqb,
                        "wob": wob, "bias": b_, "ident": ident,
                        "dmask": dmask, "kmask": kmask})
    return in_maps, npc


last_result = None


def build_compiled(ntok, n_cores=8):
    nc = bacc.Bacc("TRN2", target_bir_lowering=False, debug=False,
                   num_devices=n_cores)
    build_program2(nc, ntok)
    nc.compile()
    from concourse.bass_interp import get_hw_module
    nc.m = get_hw_module(nc.m)
    return nc


def _run(in_maps, ntok, n_cores=8):
    # NTFF tracing is unsupported under this axon client; make sure the
    # spmd runner never takes the trace path.
    os.environ["BASS_NEVER_TRACE"] = "1"
    key = (ntok, n_cores)
    if key not in _prog_cache:
        _prog_cache[key] = build_compiled(ntok, n_cores)
    nc = _prog_cache[key]
    res = bass_utils.run_bass_kernel_spmd(nc, in_maps, list(range(n_cores)))
    global last_result
    last_result = res
    return res


def make_in_maps(x, Wq, Wk, Wv, Wo, bo, n_cores=8, mmdt="f32r", fuseq=False):
    import ml_dtypes
    if mmdt == "f32r":
        xdt = np.float32
    else:
        xdt = ml_dtypes.bfloat16
    sdt = ml_dtypes.bfloat16 if fuseq else xdt
    x = np.asarray(x, dtype=np.float32)
    B, N, _ = x.shape
    npc = B * N // n_cores  # tokens per core
    halves = N // npc       # token halves per batch item
    wqt = np.ascontiguousarray(np.asarray(Wq, np.float32).T).astype(xdt)
    wkt = np.ascontiguousarray(np.asarray(Wk, np.float32).T).astype(xdt)
    wvt = np.ascontiguousarray(np.asarray(Wv, np.float32).T).astype(xdt)
    wot = np.ascontiguousarray(np.asarray(Wo, np.float32).T).astype(xdt)
    b_ = np.asarray(bo, np.float32).reshape(1, D)
    selc = np.zeros((H, NPAIR * P), dtype=sdt)
    for p in range(NPAIR):
        selc[2 * p, p * P:p * P + HD] = 1.0
        selc[2 * p + 1, p * P + HD:(p + 1) * P] = 1.0
    dmask = np.zeros((P, P), dtype=np.float32)
    dmask[:HD, :HD] = 1.0
    dmask[HD:, HD:] = 1.0
    kmask = np.zeros((P, KC * H), dtype=np.float32)
    for c in range(KC):
        kmask[:HD, c * H + 2 * c] = 1.0
        kmask[HD:, c * H + 2 * c + 1] = 1.0
    in_maps = []
    for i in range(n_cores):
        b, h = divmod(i, halves)
        xs = np.ascontiguousarray(x[b, h * npc:(h + 1) * npc, :].T).astype(xdt)
        key = "selcb" if fuseq else "selc"
        in_maps.append({"xt": xs, "wqt": wqt, "wkt": wkt, "wvt": wvt,
                        "wot": wot, "bias": b_, key: selc,
                        "dmask": dmask, "kmask": kmask})
    return in_maps, npc


_prog_cache = {}


def kernel(x, Wq, Wk, Wv, Wo, bo):
    x = np.asarray(x, dtype=np.float32)
    B, N, _ = x.shape
    n_cores = 8
    in_maps, npc = make_in_maps2(x, Wq, Wk, Wv, Wo, bo, n_cores)
    halves = N // npc
    res = _run(in_maps, npc, n_cores)
    out = np.empty((B, N, D), dtype=np.float32)
    for i in range(n_cores):
        b, h = divmod(i, halves)
        out[b, h * npc:(h + 1) * npc, :] = res.results[i]["out"]
    return out



# revision 18
# speedup vs baseline: 1.1359x; 1.0709x over previous
"""Bidirectional linear attention kernel for 8 TRN2 NeuronCores.

Sharding: core i handles batch b = i//2, token half i%2 (4096 tokens each).
Per-head KV aggregation is completed with an AllReduce of the packed
[kv | ksum] accumulator (528KB) within core pairs {0,1},{2,3},{4,5},{6,7}.

Layout strategy (per core):
  - x arrives host-transposed as xT [D, Ntok] (feature-major).
  - Phase 1: k, v computed token-major ([tok, D]) with fp32r matmuls;
    phi(k) = max(k+1, exp(min(k,0))) fused ACT+DVE; kv per head-pair via
    bf16 matmuls with a ones-column appended to v so ksum falls out of the
    same matmul (out [128, 129] per pair), accumulated into SBUF.
  - AllReduce pairs over the packed [128, 1032] kv/ksum buffer.
  - Phase 2: qT computed feature-major; numerator^T = blockdiag(kv)^T @ qT;
    denominator for all heads via blockdiag-ksum matmul -> [16, T];
    reciprocal broadcast back to 64 partitions/head with a small select
    matmul; divide; output projection token-major + bias.
"""

import os
import sys

import numpy as np

for _p in ("/opt/trn_rl_repo", "/root/.axon_site/_ro/trn_rl_repo"):
    if os.path.isdir(_p) and _p not in sys.path:
        sys.path.append(_p)

from contextlib import ExitStack

import concourse.bacc as bacc
import concourse.tile as tile
from concourse import mybir
from concourse.alu_op_type import AluOpType
from concourse import bass_utils

F32 = mybir.dt.float32
F32R = mybir.dt.float32r
BF16 = mybir.dt.bfloat16
AF = mybir.ActivationFunctionType

D = 1024        # model dim
H = 16          # heads
HD = 64         # head dim
P = 128         # partitions
NPAIR = 8       # head pairs
KC = 8          # feature chunks of 128
TC = 512        # token chunk
EPS = 1e-6
PW = 129        # pair width in the packed kv|ksum layout


def r(ap):
    return ap.bitcast(F32R)


def build_program(nc, ntok, use_cc=True, reps=1, mmdt="f32r",
                  do_p1=True, do_p2=True, abl=(), fuseq=False, rbc_gps=False,
                  atbf=False, qcp=False, kcp=False):
    abl = set(abl)
    nch = ntok // TC
    MDT = F32R if mmdt == "f32r" else BF16
    XDT = F32 if mmdt == "f32r" else BF16

    def rr(ap):
        return ap.bitcast(F32R) if mmdt == "f32r" else ap

    xt = nc.dram_tensor("xt", [D, ntok], XDT, kind="ExternalInput").ap()
    wqt = nc.dram_tensor("wqt", [D, D], XDT, kind="ExternalInput").ap()
    wkt = nc.dram_tensor("wkt", [D, D], XDT, kind="ExternalInput").ap()
    wvt = nc.dram_tensor("wvt", [D, D], XDT, kind="ExternalInput").ap()
    wot = (nc.dram_tensor("wotb", [D, D], BF16, kind="ExternalInput").ap()
           if atbf else
           nc.dram_tensor("wot", [D, D], XDT, kind="ExternalInput").ap())
    bias = nc.dram_tensor("bias", [1, D], F32, kind="ExternalInput").ap()
    out = nc.dram_tensor("out", [ntok, D], F32, kind="ExternalOutput").ap()

    cc_in = nc.dram_tensor("cc_in", [P, NPAIR * PW], F32).ap()
    cc_out = nc.dram_tensor("cc_out", [P, NPAIR * PW], F32).ap()
    QDT = BF16 if fuseq else MDT
    qsp = nc.dram_tensor("qsp", [D, ntok], BF16).ap() if fuseq else None

    with tile.TileContext(nc) as tc, ExitStack() as es:
        p_w = es.enter_context(tc.tile_pool(name="w", bufs=24 if fuseq else 16))
        p_x = es.enter_context(tc.tile_pool(name="x", bufs=9))
        p_scr = es.enter_context(tc.tile_pool(name="scr", bufs=3 if atbf else 2))
        p_qc = es.enter_context(tc.tile_pool(name="qc", bufs=3))
        p_c1 = es.enter_context(tc.tile_pool(name="c1", bufs=1))
        p_c8 = es.enter_context(tc.tile_pool(name="c8", bufs=8))

        # bias broadcast [1, D] -> [128, D]
        b_row = p_c1.tile([1, D], F32, tag="brow")
        nc.sync.dma_start(b_row[:], bias[:, :])
        bias_bc = p_c1.tile([P, D], F32, tag="bias")
        nc.gpsimd.partition_broadcast(bias_bc[:], b_row[:])

        # select matrices for the reciprocal broadcast (host-prepared constant)
        selc = nc.dram_tensor("selcb" if fuseq else "selc", [H, NPAIR * P],
                          BF16 if fuseq else XDT, kind="ExternalInput").ap()
        sel_t = p_c1.tile([H, NPAIR * P], QDT, tag="sel")
        nc.sync.dma_start(sel_t[:], selc[:, :] if fuseq else rr(selc[:, :]))
        sel = [sel_t[:, p * P:(p + 1) * P] for p in range(NPAIR)]

        # masks for block-diag kv / ksum lhsT construction
        dmask = nc.dram_tensor("dmask", [P, P], F32, kind="ExternalInput").ap()
        dmask_t = p_c1.tile([P, P], F32, tag="dmask")
        nc.sync.dma_start(dmask_t[:], dmask[:, :])
        kmask = nc.dram_tensor("kmask", [P, KC * H], F32,
                               kind="ExternalInput").ap()
        kmask_t = p_c1.tile([P, KC * H], F32, tag="kmask")
        nc.sync.dma_start(kmask_t[:], kmask[:, :])

        # packed kv | ksum accumulator
        kvks = p_c1.tile([P, NPAIR * PW], F32, tag="kvks")
        kvks3 = kvks.rearrange("p (a b) -> p a b", b=PW)

        def load_xt(j):
            ts = []
            for k in range(KC):
                t = p_x.tile([P, TC], MDT, tag="xt")
                nc.sync.dma_start(
                    t[:], rr(xt[k * P:(k + 1) * P, j * TC:(j + 1) * TC]))
                ts.append(t)
            return ts

        for _rep in range(reps):
            # ---- weights (phase 1) ----
            wkt_t = []
            wvt_t = []
            wq1_t = []
            if do_p1:
                for k in range(KC):
                    t = p_w.tile([P, D], MDT, tag="w")
                    nc.sync.dma_start(t[:], rr(wkt[k * P:(k + 1) * P, :]))
                    wkt_t.append(t)
                for k in range(KC):
                    t = p_w.tile([P, D], MDT, tag="w")
                    nc.sync.dma_start(t[:], rr(wvt[k * P:(k + 1) * P, :]))
                    wvt_t.append(t)
                if fuseq:
                    for k in range(KC):
                        t = p_w.tile([P, D], MDT, tag="w")
                        nc.sync.dma_start(t[:], rr(wqt[k * P:(k + 1) * P, :]))
                        wq1_t.append(t)
            nc.vector.memset(kvks[:], 0.0)

            # ================= phase 1: k, v, kv, ksum =================
            with ExitStack() as es1:
                ps_k = es1.enter_context(tc.tile_pool(name="psk", bufs=2, space="PSUM"))
                ps_v = es1.enter_context(tc.tile_pool(name="psv", bufs=2, space="PSUM"))
                ps_kv = es1.enter_context(tc.tile_pool(name="pskv", bufs=2, space="PSUM"))
                p_kphi = es1.enter_context(tc.tile_pool(name="kphi", bufs=6))
                p_v = es1.enter_context(tc.tile_pool(name="vsb", bufs=6))
                if fuseq:
                    ps_q1 = es1.enter_context(
                        tc.tile_pool(name="psq1", bufs=2, space="PSUM"))
                    p_q1 = es1.enter_context(tc.tile_pool(name="q1", bufs=3))

                for j in range(nch if do_p1 else 0):
                    xts = load_xt(j)
                    kphis = []
                    vsbs = []
                    for m in range(4):
                        xm = [xts[k][:, m * P:(m + 1) * P] for k in range(KC)]
                        # k projection + phi -> bf16 [128, 1024]
                        kph = p_kphi.tile([P, D], BF16, tag="kphi")
                        for n in range(2):
                            kp = ps_k.tile([P, TC], F32, tag="kp")
                            for k in range(KC):
                                nc.tensor.matmul(
                                    kp[:], xm[k], wkt_t[k][:, n * TC:(n + 1) * TC],
                                    start=(k == 0), stop=(k == KC - 1))
                            if "phi" in abl:
                                nc.scalar.activation(
                                    kph[:, n * TC:(n + 1) * TC], kp[:], AF.Copy)
                            elif kcp:
                                kc_ = p_qc.tile([P, TC], F32, tag="kc")
                                nc.scalar.activation(kc_[:], kp[:], AF.Copy)
                                mx = p_scr.tile([P, TC], F32, tag="mx")
                                nc.vector.tensor_scalar_min(mx[:], kc_[:], 0.0)
                                ex = p_scr.tile([P, TC], F32, tag="ex")
                                nc.scalar.activation(ex[:], mx[:], AF.Exp)
                                nc.vector.scalar_tensor_tensor(
                                    kph[:, n * TC:(n + 1) * TC], kc_[:], 1.0,
                                    ex[:], AluOpType.add, AluOpType.max)
                            else:
                                mx = p_scr.tile([P, TC], F32, tag="mx")
                                nc.vector.tensor_scalar_min(mx[:], kp[:], 0.0)
                                ex = p_scr.tile([P, TC], F32, tag="ex")
                                nc.scalar.activation(ex[:], mx[:], AF.Exp)
                                nc.vector.scalar_tensor_tensor(
                                    kph[:, n * TC:(n + 1) * TC], kp[:], 1.0, ex[:],
                                    AluOpType.add, AluOpType.max)
                        kphis.append(kph)
                        # v projection -> bf16 [128, 1032] interleaved with ones cols
                        vsb = p_v.tile([P, NPAIR * PW], BF16, tag="vsb")
                        vsb3 = vsb.rearrange("p (a b) -> p a b", b=PW)
                        for n in range(2):
                            vp = ps_v.tile([P, TC], F32, tag="vp")
                            for k in range(KC):
                                nc.tensor.matmul(
                                    vp[:], xm[k], wvt_t[k][:, n * TC:(n + 1) * TC],
                                    start=(k == 0), stop=(k == KC - 1))
                            if "vcopy" not in abl:
                                nc.scalar.activation(
                                    vsb3[:, 4 * n:4 * n + 4, 0:P],
                                    vp.rearrange("p (a b) -> p a b", b=P), AF.Copy)
                        if "vcopy" not in abl:
                            nc.vector.memset(vsb3[:, :, P:PW], 1.0)
                        else:
                            nc.vector.memset(vsb[:], 1.0)
                        vsbs.append(vsb)
                    # fused q projection + phi + spill to DRAM (bf16)
                    if fuseq:
                        for c in range(KC):
                            qp1 = ps_q1.tile([P, TC], F32, tag="qp1")
                            for k in range(KC):
                                nc.tensor.matmul(
                                    qp1[:], wq1_t[k][:, c * P:(c + 1) * P],
                                    xts[k][:],
                                    start=(k == 0), stop=(k == KC - 1))
                            q1 = p_q1.tile([P, TC], BF16, tag="q1")
                            if "phi" in abl:
                                nc.scalar.activation(q1[:], qp1[:], AF.Copy)
                            else:
                                mx = p_scr.tile([P, TC], F32, tag="mx")
                                nc.vector.tensor_scalar_min(mx[:], qp1[:], 0.0)
                                ex = p_scr.tile([P, TC], F32, tag="ex")
                                nc.scalar.activation(ex[:], mx[:], AF.Exp)
                                nc.vector.scalar_tensor_tensor(
                                    q1[:], qp1[:], 1.0, ex[:], AluOpType.add,
                                    AluOpType.max)
                            nc.sync.dma_start(
                                qsp[c * P:(c + 1) * P, j * TC:(j + 1) * TC],
                                q1[:])
                    # kv + ksum accumulation per pair
                    for p in range(NPAIR if "kv" not in abl else 0):
                        kvp = ps_kv.tile([P, PW], F32, tag="kvp")
                        for m in range(4):
                            nc.tensor.matmul(
                                kvp[:], kphis[m][:, p * P:(p + 1) * P],
                                vsbs[m].rearrange("p (a b) -> p a b", b=PW)[:, p, :],
                                start=(m == 0), stop=(m == 3))
                        nc.vector.tensor_tensor(
                            kvks3[:, p, :], kvp[:], kvks3[:, p, :], AluOpType.add)

            # ================= allreduce within pairs =================
            nc.sync.dma_start(cc_in[:, :], kvks[:])
            if use_cc:
                nc.gpsimd.collective_compute(
                    "AllReduce", AluOpType.add,
                    replica_groups=[[0, 1], [2, 3], [4, 5], [6, 7]],
                    ins=[cc_in[:, :]], outs=[cc_out[:, :]])
            else:
                nc.sync.dma_start(cc_out[:, :], cc_in[:, :])
            red = p_c1.tile([P, NPAIR * PW], F32, tag="red")
            nc.sync.dma_start(red[:], cc_out[:, :])
            red3 = red.rearrange("p (a b) -> p a b", b=PW)

            # block-diag kv lhsT per pair (diag-mask multiply), block ksum lhsT
            # per chunk (per-partition scalar multiply against a column mask).
            kvbd = []
            ksbd = []
            for p in range(NPAIR):
                t = p_c8.tile([P, P], QDT, tag="kvbd")
                nc.vector.tensor_tensor(t[:], red3[:, p, 0:P], dmask_t[:],
                                        AluOpType.mult)
                kvbd.append(t)
            for c in range(KC):
                t = p_c8.tile([P, H], QDT, tag="ksbd")
                nc.vector.tensor_scalar(
                    t[:], kmask_t[:, c * H:(c + 1) * H], red3[:, c, P:PW], None,
                    AluOpType.mult)
                ksbd.append(t)

            # ---- weights (phase 2) ----
            wqt_t = []
            wot_t = []
            if do_p2:
                for k in range(KC):
                    t = p_w.tile([P, D], MDT, tag="w")
                    nc.sync.dma_start(t[:], rr(wqt[k * P:(k + 1) * P, :]))
                    wqt_t.append(t)
                for k in range(KC):
                    if atbf:
                        t = p_w.tile([P, D], BF16, tag="w")
                        nc.sync.dma_start(t[:], wot[k * P:(k + 1) * P, :])
                    else:
                        t = p_w.tile([P, D], MDT, tag="w")
                        nc.sync.dma_start(t[:], rr(wot[k * P:(k + 1) * P, :]))
                    wot_t.append(t)

            # ================= phase 2: q, numerator, denom, y =================
            with ExitStack() as es2:
                ps_q = es2.enter_context(tc.tile_pool(name="psq", bufs=2, space="PSUM"))
                ps_num = es2.enter_context(tc.tile_pool(name="psnum", bufs=2, space="PSUM"))
                ps_rbc = es2.enter_context(tc.tile_pool(name="psrbc", bufs=2, space="PSUM"))
                ps_y = es2.enter_context(tc.tile_pool(name="psy", bufs=2, space="PSUM"))
                p_qt = es2.enter_context(tc.tile_pool(name="qt", bufs=16))
                p_at = es2.enter_context(tc.tile_pool(name="at", bufs=16 if atbf else 11))
                p_rbc = es2.enter_context(tc.tile_pool(name="rbc", bufs=3))
                p_y = es2.enter_context(tc.tile_pool(name="ysb", bufs=3))
                p_dn = es2.enter_context(tc.tile_pool(name="dn", bufs=1))

                def emit_qproj(j):
                    xts = load_xt(j)
                    qts = []
                    for c in range(KC):
                        qp = ps_q.tile([P, TC], F32, tag="qp")
                        for k in range(KC):
                            nc.tensor.matmul(
                                qp[:], wqt_t[k][:, c * P:(c + 1) * P], xts[k][:],
                                start=(k == 0), stop=(k == KC - 1))
                        qt_ = p_qt.tile([P, TC], MDT, tag="qt")
                        if "phi" in abl:
                            nc.scalar.activation(qt_[:], qp[:], AF.Copy)
                        elif qcp:
                            # release the PSUM tile with one ACT copy, then
                            # run the phi chain from SBUF so the PE is not
                            # gated on the full DVE/ACT consumer chain
                            qc = p_qc.tile([P, TC], F32, tag="qc")
                            nc.scalar.activation(qc[:], qp[:], AF.Copy)
                            mx = p_scr.tile([P, TC], F32, tag="mx")
                            nc.vector.tensor_scalar_min(mx[:], qc[:], 0.0)
                            ex = p_scr.tile([P, TC], F32, tag="ex")
                            nc.scalar.activation(ex[:], mx[:], AF.Exp)
                            nc.vector.scalar_tensor_tensor(
                                qt_[:], qc[:], 1.0, ex[:], AluOpType.add,
                                AluOpType.max)
                        else:
                            mx = p_scr.tile([P, TC], F32, tag="mx")
                            nc.vector.tensor_scalar_min(mx[:], qp[:], 0.0)
                            ex = p_scr.tile([P, TC], F32, tag="ex")
                            nc.scalar.activation(ex[:], mx[:], AF.Exp)
                            nc.vector.scalar_tensor_tensor(
                                qt_[:], qp[:], 1.0, ex[:], AluOpType.add,
                                AluOpType.max)
                        qts.append(qt_)
                    return qts

                def emit_y(j, attns):
                    if "ycompute" in abl:
                        return
                    for m in range(4):
                        for n in range(2):
                            yp = ps_y.tile([P, TC], F32, tag="yp")
                            for k in range(KC):
                                nc.tensor.matmul(
                                    yp[:], attns[k][:, m * P:(m + 1) * P],
                                    wot_t[k][:, n * TC:(n + 1) * TC],
                                    start=(k == 0), stop=(k == KC - 1))
                            ysb = p_y.tile([P, TC], F32, tag="ysb")
                            nc.vector.tensor_tensor(
                                ysb[:], yp[:], bias_bc[:, n * TC:(n + 1) * TC],
                                AluOpType.add)
                            row0 = j * TC + m * P
                            nc.sync.dma_start(
                                out[row0:row0 + P, n * TC:(n + 1) * TC], ysb[:])

                def load_qt(j):
                    ts = []
                    for c in range(KC):
                        t = p_x.tile([P, TC], BF16, tag="qtl")
                        nc.sync.dma_start(
                            t[:], qsp[c * P:(c + 1) * P, j * TC:(j + 1) * TC])
                        ts.append(t)
                    return ts

                if fuseq:
                    qtiles = {0: load_qt(0)} if do_p2 else {}
                else:
                    qtiles = {0: emit_qproj(0)} if do_p2 else {}
                attn_prev = None
                for j in range(nch if do_p2 else 0):
                    if j + 1 < nch:
                        qtiles[j + 1] = load_qt(j + 1) if fuseq else emit_qproj(j + 1)
                    qts = qtiles.pop(j)
                    if "numrbc" in abl:
                        attn_prev_new = qts
                        if attn_prev is not None:
                            emit_y(j - 1, attn_prev)
                        attn_prev = attn_prev_new
                        continue
                    # denominator for all heads: [16, TC]
                    dps_full = ps_rbc.tile([P, TC], F32, tag="rbps")
                    dps = dps_full[0:H, :]
                    for c in range(KC):
                        nc.tensor.matmul(dps[:], ksbd[c][:], qts[c][:],
                                         start=(c == 0), stop=(c == KC - 1))
                    if attn_prev is not None:
                        emit_y(j - 1, attn_prev)
                    dsb = p_dn.tile([H, TC], F32, tag="dsb")
                    nc.vector.tensor_scalar_add(dsb[:], dps[:], EPS)
                    rcp = p_dn.tile([H, TC], F32 if rbc_gps else QDT, tag="rcp")
                    with nc.allow_low_precision(reason="f32r rounding of recip"):
                        nc.vector.reciprocal(rcp[:], dsb[:])
                    attns = []
                    for p in range(NPAIR):
                        np_ = ps_num.tile([P, TC], F32, tag="nps")
                        nc.tensor.matmul(np_[:], kvbd[p][:], qts[p][:],
                                         start=True, stop=True)
                        rbs = p_rbc.tile([P, TC], F32, tag="rbs")
                        if rbc_gps:
                            nc.gpsimd.partition_broadcast(
                                rbs[0:HD, :], rcp[2 * p:2 * p + 1, :])
                            nc.gpsimd.partition_broadcast(
                                rbs[HD:P, :], rcp[2 * p + 1:2 * p + 2, :])
                        else:
                            rb = ps_rbc.tile([P, TC], F32, tag="rbps")
                            nc.tensor.matmul(rb[:], sel[p], rcp[:],
                                             start=True, stop=True)
                            nc.scalar.activation(rbs[:], rb[:], AF.Copy)
                        at = p_at.tile([P, TC], BF16 if atbf else MDT, tag="at")
                        nc.vector.tensor_tensor(at[:], np_[:], rbs[:], AluOpType.mult)
                        attns.append(at)
                    attn_prev = attns
                if do_p2:
                    emit_y(nch - 1, attn_prev)

    return nc


def build_program2(nc, ntok, reps=1, use_cc=True, do_p1=True, do_p2=True,
                   abl=(), qpre=4, ccbf=True, rev=True, ydma_act=True):
    """v2: all-bf16 matmuls; numerator folded into the output projection via
    M = blockdiag(kv) @ Wo^T; reciprocal applied to q via Pool broadcast;
    all four weight matrices prefetched at rep start (bf16 halves SBUF and
    DMA so they fit resident simultaneously); q-projection runs `qpre`
    chunks ahead so the pair AllReduce hides behind PE work.
    """
    abl = set(abl)
    nch = ntok // TC

    xt = nc.dram_tensor("xt", [D, ntok], BF16, kind="ExternalInput").ap()
    wkb = nc.dram_tensor("wkb", [D, D], BF16, kind="ExternalInput").ap()
    wvb = nc.dram_tensor("wvb", [D, D], BF16, kind="ExternalInput").ap()
    wqb = nc.dram_tensor("wqb", [D, D], BF16, kind="ExternalInput").ap()
    wob = nc.dram_tensor("wob", [D, D], BF16, kind="ExternalInput").ap()
    bias = nc.dram_tensor("bias", [1, D], F32, kind="ExternalInput").ap()
    ident = nc.dram_tensor("ident", [P, P], BF16, kind="ExternalInput").ap()
    dmask = nc.dram_tensor("dmask", [P, P], F32, kind="ExternalInput").ap()
    kmask = nc.dram_tensor("kmask", [P, KC * H], F32,
                           kind="ExternalInput").ap()
    out = nc.dram_tensor("out", [ntok, D], F32, kind="ExternalOutput").ap()

    CDT = BF16 if ccbf else F32
    cc_in = nc.dram_tensor("cc_in", [P, NPAIR * PW], CDT).ap()
    cc_out = nc.dram_tensor("cc_out", [P, NPAIR * PW], CDT).ap()

    with tile.TileContext(nc) as tc, ExitStack() as es:
        p_w = es.enter_context(tc.tile_pool(name="w", bufs=32))
        p_x = es.enter_context(tc.tile_pool(name="x", bufs=9))
        p_scr = es.enter_context(tc.tile_pool(name="scr", bufs=2))
        p_qc = es.enter_context(tc.tile_pool(name="qc", bufs=3))
        p_qt = es.enter_context(tc.tile_pool(name="qt", bufs=32))
        p_kphi = es.enter_context(tc.tile_pool(name="kphi", bufs=6))
        p_v = es.enter_context(tc.tile_pool(name="vsb", bufs=6))
        p_m = es.enter_context(tc.tile_pool(name="m", bufs=8))
        p_kv8 = es.enter_context(tc.tile_pool(name="kv8", bufs=8))
        p_rbs = es.enter_context(tc.tile_pool(name="rbs", bufs=5))
        p_dn = es.enter_context(tc.tile_pool(name="dn", bufs=2))
        p_rfl = es.enter_context(tc.tile_pool(name="rfl", bufs=1))
        p_y = es.enter_context(tc.tile_pool(name="ysb", bufs=3))
        p_c1 = es.enter_context(tc.tile_pool(name="c1", bufs=1))

        # constants
        b_row = p_c1.tile([1, D], F32, tag="brow")
        nc.sync.dma_start(b_row[:], bias[:, :])
        bias_bc = p_c1.tile([P, D], F32, tag="bias")
        nc.gpsimd.partition_broadcast(bias_bc[:], b_row[:])
        ident_t = p_c1.tile([P, P], BF16, tag="ident")
        nc.sync.dma_start(ident_t[:], ident[:, :])
        dmask_t = p_c1.tile([P, P], F32, tag="dmask")
        nc.sync.dma_start(dmask_t[:], dmask[:, :])
        kmask_t = p_c1.tile([P, KC * H], F32, tag="kmask")
        nc.sync.dma_start(kmask_t[:], kmask[:, :])

        kvks = p_c1.tile([P, NPAIR * PW], F32, tag="kvks")
        kvks3 = kvks.rearrange("p (a b) -> p a b", b=PW)

        def load_w(src):
            ts = []
            for k in range(KC):
                t = p_w.tile([P, D], BF16, tag="w")
                nc.sync.dma_start(t[:], src[k * P:(k + 1) * P, :])
                ts.append(t)
            return ts

        def load_xt(j):
            ts = []
            for k in range(KC):
                t = p_x.tile([P, TC], BF16, tag="xt")
                nc.sync.dma_start(
                    t[:], xt[k * P:(k + 1) * P, j * TC:(j + 1) * TC])
                ts.append(t)
            return ts

        def phi_from(psum, dst, via):
            # dst = max(src + 1, exp(min(src, 0))), src read once into `via`
            nc.scalar.activation(via[:], psum[:], AF.Copy)
            mx = p_scr.tile([P, TC], BF16, tag="mx")
            nc.vector.tensor_scalar_min(mx[:], via[:], 0.0)
            ex = p_scr.tile([P, TC], BF16, tag="ex")
            nc.scalar.activation(ex[:], mx[:], AF.Exp)
            nc.vector.scalar_tensor_tensor(
                dst, via[:], 1.0, ex[:], AluOpType.add, AluOpType.max)

        for _rep in range(reps):
            # interleave wk with x(0) so the first k-proj group is DMA-paced
            # rather than blocked on the full weight prefetch
            wkt = []
            xts0 = []
            if do_p1:
                for k in range(KC):
                    t = p_w.tile([P, D], BF16, tag="w")
                    nc.sync.dma_start(t[:], wkb[k * P:(k + 1) * P, :])
                    wkt.append(t)
                    tx = p_x.tile([P, TC], BF16, tag="xt")
                    nc.sync.dma_start(tx[:], xt[k * P:(k + 1) * P, 0:TC])
                    xts0.append(tx)
            wvt = load_w(wvb) if do_p1 else []
            wqt = load_w(wqb) if do_p2 else []
            wot = load_w(wob) if do_p2 else []
            nc.vector.memset(kvks[:], 0.0)

            # ================= phase 1: k, v, kv|ksum =================
            with ExitStack() as es1:
                ps_k = es1.enter_context(
                    tc.tile_pool(name="psk", bufs=2, space="PSUM"))
                ps_v = es1.enter_context(
                    tc.tile_pool(name="psv", bufs=2, space="PSUM"))
                ps_kv = es1.enter_context(
                    tc.tile_pool(name="pskv", bufs=2, space="PSUM"))

                for j in range(nch if do_p1 else 0):
                    xts = xts0 if j == 0 else load_xt(j)
                    kphis = []
                    vsbs = []
                    for m in range(4):
                        xm = [xts[k][:, m * P:(m + 1) * P] for k in range(KC)]
                        kph = p_kphi.tile([P, D], BF16, tag="kphi")
                        for n in range(2):
                            kp = ps_k.tile([P, TC], F32, tag="kp")
                            for k in range(KC):
                                nc.tensor.matmul(
                                    kp[:], xm[k],
                                    wkt[k][:, n * TC:(n + 1) * TC],
                                    start=(k == 0), stop=(k == KC - 1))
                            if "phi" in abl:
                                nc.scalar.activation(
                                    kph[:, n * TC:(n + 1) * TC], kp[:],
                                    AF.Copy)
                            else:
                                kc_ = p_qc.tile([P, TC], BF16, tag="qc")
                                phi_from(kp, kph[:, n * TC:(n + 1) * TC], kc_)
                        kphis.append(kph)
                        vsb = p_v.tile([P, NPAIR * PW], BF16, tag="vsb")
                        vsb3 = vsb.rearrange("p (a b) -> p a b", b=PW)
                        for n in range(2):
                            vp = ps_v.tile([P, TC], F32, tag="vp")
                            for k in range(KC):
                                nc.tensor.matmul(
                                    vp[:], xm[k],
                                    wvt[k][:, n * TC:(n + 1) * TC],
                                    start=(k == 0), stop=(k == KC - 1))
                            nc.scalar.activation(
                                vsb3[:, 4 * n:4 * n + 4, 0:P],
                                vp.rearrange("p (a b) -> p a b", b=P),
                                AF.Copy)
                        nc.vector.memset(vsb3[:, :, P:PW], 1.0)
                        vsbs.append(vsb)
                    for p in range(NPAIR if "kv" not in abl else 0):
                        kvp = ps_kv.tile([P, PW], F32, tag="kvp")
                        for m in range(4):
                            nc.tensor.matmul(
                                kvp[:], kphis[m][:, p * P:(p + 1) * P],
                                vsbs[m].rearrange(
                                    "p (a b) -> p a b", b=PW)[:, p, :],
                                start=(m == 0), stop=(m == 3))
                        nc.vector.tensor_tensor(
                            kvks3[:, p, :], kvp[:], kvks3[:, p, :],
                            AluOpType.add)

            # ================= allreduce within pairs =================
            if ccbf:
                kvksb = p_c1.tile([P, NPAIR * PW], BF16, tag="kvksb")
                nc.vector.tensor_copy(kvksb[:], kvks[:])
                nc.sync.dma_start(cc_in[:, :], kvksb[:])
            else:
                nc.sync.dma_start(cc_in[:, :], kvks[:])
            if use_cc:
                nc.gpsimd.collective_compute(
                    "AllReduce", AluOpType.add,
                    replica_groups=[[0, 1], [2, 3], [4, 5], [6, 7]],
                    ins=[cc_in[:, :]], outs=[cc_out[:, :]])
            else:
                nc.sync.dma_start(cc_out[:, :], cc_in[:, :])
            red = p_c1.tile([P, NPAIR * PW], CDT, tag="red")
            nc.sync.dma_start(red[:], cc_out[:, :])
            red3 = red.rearrange("p (a b) -> p a b", b=PW)

            # ================= phase 2: q, den, y = q' @ M =================
            with ExitStack() as es2:
                ps_q = es2.enter_context(
                    tc.tile_pool(name="psq", bufs=2, space="PSUM"))
                ps_dn = es2.enter_context(
                    tc.tile_pool(name="psdn", bufs=2, space="PSUM"))
                ps_y = es2.enter_context(
                    tc.tile_pool(name="psy", bufs=2, space="PSUM"))
                ps_mt = es2.enter_context(
                    tc.tile_pool(name="psmt", bufs=1, space="PSUM"))

                def emit_qproj(j, xts=None):
                    if xts is None:
                        xts = load_xt(j)
                    qts = []
                    for c in range(KC):
                        qp = ps_q.tile([P, TC], F32, tag="qp")
                        for k in range(KC):
                            nc.tensor.matmul(
                                qp[:], wqt[k][:, c * P:(c + 1) * P], xts[k][:],
                                start=(k == 0), stop=(k == KC - 1))
                        qt_ = p_qt.tile([P, TC], BF16, tag="qt")
                        if "phi" in abl:
                            nc.scalar.activation(qt_[:], qp[:], AF.Copy)
                        else:
                            qc = p_qc.tile([P, TC], BF16, tag="qc")
                            phi_from(qp, qt_[:], qc)
                        qts.append(qt_)
                    return qts

                def emit_den(j, qts):
                    # denominator for all heads -> reciprocal on partition 0
                    dps_full = ps_dn.tile([P, TC], F32, tag="dn")
                    dps = dps_full[0:H, :]
                    for c in range(KC):
                        nc.tensor.matmul(dps[:], ksbd[c][:], qts[c][:],
                                         start=(c == 0), stop=(c == KC - 1))
                    dsb = p_dn.tile([H, TC], F32, tag="dsb")
                    nc.vector.tensor_scalar_add(dsb[:], dps[:], EPS)
                    rcp = p_dn.tile([H, TC], BF16, tag="rcp")
                    with nc.allow_low_precision(reason="bf16 recip"):
                        nc.vector.reciprocal(rcp[:], dsb[:])
                    rfl = p_rfl.tile([1, H * TC], BF16, tag="rfl")
                    rfl3 = rfl.rearrange("p (a b) -> p a b", b=TC)
                    nc.sync.dma_start(rfl3[:, :, :], rcp[:, :])
                    if "scale" in abl:
                        return
                    for c in range(KC):
                        rbe = p_rbs.tile([P, TC], BF16, tag="rbs")
                        nc.gpsimd.partition_broadcast(
                            rbe[:, :], rfl3[:, 2 * c, :])
                        rbo = p_rbs.tile([P, TC], BF16, tag="rbs")
                        nc.gpsimd.partition_broadcast(
                            rbo[:, :], rfl3[:, 2 * c + 1, :])
                        nc.vector.tensor_tensor(
                            qts[c][0:HD, :], qts[c][0:HD, :], rbe[0:HD, :],
                            AluOpType.mult)
                        nc.vector.tensor_tensor(
                            qts[c][HD:P, :], qts[c][HD:P, :], rbo[HD:P, :],
                            AluOpType.mult)

                def emit_y(j, qts):
                    if "y" in abl:
                        return
                    for m in range(4):
                        for n in range(2):
                            yp = ps_y.tile([P, TC], F32, tag="yp")
                            for c in range(KC):
                                nc.tensor.matmul(
                                    yp[:], qts[c][:, m * P:(m + 1) * P],
                                    m_t[c][:, n * TC:(n + 1) * TC],
                                    start=(c == 0), stop=(c == KC - 1))
                            ysb = p_y.tile([P, TC], F32, tag="ysb")
                            nc.vector.tensor_tensor(
                                ysb[:], yp[:],
                                bias_bc[:, n * TC:(n + 1) * TC],
                                AluOpType.add)
                            row0 = j * TC + m * P
                            eng = nc.scalar if ydma_act else nc.sync
                            eng.dma_start(
                                out[row0:row0 + P, n * TC:(n + 1) * TC],
                                ysb[:])

                if do_p2:
                    # reversed chunk order reuses phase-1's resident x(last)
                    order = list(range(nch))[::-1] if rev else list(range(nch))
                    qtiles = {}
                    for idx in range(min(qpre, nch)):
                        j = order[idx]
                        xts_j = None
                        if rev and do_p1 and idx == 0:
                            xts_j = xts
                        qtiles[j] = emit_qproj(j, xts_j)

                    # M = blockdiag(kv) @ Wo^T  (per pair: transpose + 2 mm)
                    m_t = []
                    ksbd = []
                    for p in range(NPAIR):
                        kvbd = p_kv8.tile([P, P], BF16, tag="kvbd")
                        nc.vector.tensor_tensor(
                            kvbd[:], red3[:, p, 0:P], dmask_t[:],
                            AluOpType.mult)
                        tp = ps_mt.tile([P, P], BF16, tag="tp")
                        nc.tensor.transpose(tp[:], kvbd[:], ident_t[:])
                        kvt = p_kv8.tile([P, P], BF16, tag="kvt")
                        nc.scalar.activation(kvt[:], tp[:], AF.Copy)
                        mt = p_m.tile([P, D], BF16, tag="m")
                        for n in range(2):
                            mm = ps_mt.tile([P, TC], F32, tag="mm")
                            nc.tensor.matmul(mm[:], kvt[:],
                                             wot[p][:, n * TC:(n + 1) * TC],
                                             start=True, stop=True)
                            nc.scalar.activation(
                                mt[:, n * TC:(n + 1) * TC], mm[:], AF.Copy)
                        m_t.append(mt)
                        t = p_kv8.tile([P, H], BF16, tag="ksbd")
                        if ccbf:
                            ksf = p_kv8.tile([P, 1], F32, tag="ksf")
                            nc.vector.tensor_copy(ksf[:], red3[:, p, P:PW])
                            nc.vector.tensor_scalar(
                                t[:], kmask_t[:, p * H:(p + 1) * H],
                                ksf[:], None, AluOpType.mult)
                        else:
                            nc.vector.tensor_scalar(
                                t[:], kmask_t[:, p * H:(p + 1) * H],
                                red3[:, p, P:PW], None, AluOpType.mult)
                        ksbd.append(t)

                    emit_den(order[0], qtiles[order[0]])
                    for idx, j in enumerate(order):
                        if idx + 1 < nch:
                            jn = order[idx + 1]
                            emit_den(jn, qtiles[jn])
                        emit_y(j, qtiles.pop(j))
                        if idx + qpre < nch:
                            jq = order[idx + qpre]
                            qtiles[jq] = emit_qproj(jq)

    return nc


def make_in_maps2(x, Wq, Wk, Wv, Wo, bo, n_cores=8):
    import ml_dtypes
    bf = ml_dtypes.bfloat16
    x = np.asarray(x, dtype=np.float32)
    B, N, _ = x.shape
    npc = B * N // n_cores
    halves = N // npc
    wkb = np.ascontiguousarray(np.asarray(Wk, np.float32).T).astype(bf)
    wvb = np.ascontiguousarray(np.asarray(Wv, np.float32).T).astype(bf)
    wqb = np.ascontiguousarray(np.asarray(Wq, np.float32).T).astype(bf)
    wob = np.ascontiguousarray(np.asarray(Wo, np.float32).T).astype(bf)
    b_ = np.asarray(bo, np.float32).reshape(1, D)
    ident = np.eye(P, dtype=bf)
    dmask = np.zeros((P, P), dtype=np.float32)
    dmask[:HD, :HD] = 1.0
    dmask[HD:, HD:] = 1.0
    kmask = np.zeros((P, KC * H), dtype=np.float32)
    for c in range(KC):
        kmask[:HD, c * H + 2 * c] = 1.0
        kmask[HD:, c * H + 2 * c + 1] = 1.0
    in_maps = []
    for i in range(n_cores):
        b, h = divmod(i, halves)
        xs = np.ascontiguousarray(x[b, h * npc:(h + 1) * npc, :].T).astype(bf)
        in_maps.append({"xt": xs, "wkb": wkb, "wvb": wvb, "wqb": wqb,
                        "wob": wob, "bias": b_, "ident": ident,
                        "dmask": dmask, "kmask": kmask})
    return in_maps, npc


last_result = None


def build_compiled(ntok, n_cores=8):
    nc = bacc.Bacc("TRN2", target_bir_lowering=False, debug=False,
                   num_devices=n_cores)
    build_program2(nc, ntok)
    nc.compile()
    from concourse.bass_interp import get_hw_module
    nc.m = get_hw_module(nc.m)
    return nc


def _run(in_maps, ntok, n_cores=8):
    # NTFF tracing is unsupported under this axon client; make sure the
    # spmd runner never takes the trace path.
    os.environ["BASS_NEVER_TRACE"] = "1"
    key = (ntok, n_cores)
    if key not in _prog_cache:
        _prog_cache[key] = build_compiled(ntok, n_cores)
    nc = _prog_cache[key]
    res = bass_utils.run_bass_kernel_spmd(nc, in_maps, list(range(n_cores)))
    global last_result
    last_result = res
    return res


def make_in_maps(x, Wq, Wk, Wv, Wo, bo, n_cores=8, mmdt="f32r", fuseq=False):
    import ml_dtypes
    if mmdt == "f32r":
        xdt = np.float32
    else:
        xdt = ml_dtypes.bfloat16
    sdt = ml_dtypes.bfloat16 if fuseq else xdt
    x = np.asarray(x, dtype=np.float32)
    B, N, _ = x.shape
    npc = B * N // n_cores  # tokens per core
    halves = N // npc       # token halves per batch item
    wqt = np.ascontiguousarray(np.asarray(Wq, np.float32).T).astype(xdt)
    wkt = np.ascontiguousarray(np.asarray(Wk, np.float32).T).astype(xdt)
    wvt = np.ascontiguousarray(np.asarray(Wv, np.float32).T).astype(xdt)
    wot = np.ascontiguousarray(np.asarray(Wo, np.float32).T).astype(xdt)
    b_ = np.asarray(bo, np.float32).reshape(1, D)
    selc = np.zeros((H, NPAIR * P), dtype=sdt)
    for p in range(NPAIR):
        selc[2 * p, p * P:p * P + HD] = 1.0
        selc[2 * p + 1, p * P + HD:(p + 1) * P] = 1.0
    dmask = np.zeros((P, P), dtype=np.float32)
    dmask[:HD, :HD] = 1.0
    dmask[HD:, HD:] = 1.0
    kmask = np.zeros((P, KC * H), dtype=np.float32)
    for c in range(KC):
        kmask[:HD, c * H + 2 * c] = 1.0
        kmask[HD:, c * H + 2 * c + 1] = 1.0
    in_maps = []
    for i in range(n_cores):
        b, h = divmod(i, halves)
        xs = np.ascontiguousarray(x[b, h * npc:(h + 1) * npc, :].T).astype(xdt)
        key = "selcb" if fuseq else "selc"
        in_maps.append({"xt": xs, "wqt": wqt, "wkt": wkt, "wvt": wvt,
                        "wot": wot, "bias": b_, key: selc,
                        "dmask": dmask, "kmask": kmask})
    return in_maps, npc


_prog_cache = {}


def kernel(x, Wq, Wk, Wv, Wo, bo):
    x = np.asarray(x, dtype=np.float32)
    B, N, _ = x.shape
    n_cores = 8
    in_maps, npc = make_in_maps2(x, Wq, Wk, Wv, Wo, bo, n_cores)
    halves = N // npc
    res = _run(in_maps, npc, n_cores)
    out = np.empty((B, N, D), dtype=np.float32)
    for i in range(n_cores):
        b, h = divmod(i, halves)
        out[b, h * npc:(h + 1) * npc, :] = res.results[i]["out"]
    return out



# revision 46
# speedup vs baseline: 1.1915x; 1.0489x over previous
"""Bidirectional linear attention kernel for 8 TRN2 NeuronCores.

Sharding: core i handles batch b = i//2, token half i%2 (4096 tokens each).
Per-head KV aggregation is completed with an AllReduce of the packed
[kv | ksum] accumulator (bf16, 264KB) within core pairs {0,1},...,{6,7}.

v2 layout (all-bf16 matmuls; build_program2 is the active builder):
  - x arrives host-transposed as xT [D, Ntok] bf16 (feature-major); all
    four weight matrices are bf16 and prefetched at rep start (they fit
    SBUF simultaneously, so the phase boundary has no weight-load stall).
  - Phase 1: k, v computed token-major; phi = max(x+1, exp(min(x, 0)))
    via ACT+DVE; kv per head-pair via bf16 matmuls with a ones-column
    appended to v so ksum falls out of the same matmul ([128, 129] per
    pair), accumulated in SBUF f32, converted bf16 for the AllReduce.
  - Phase 2 (chunks in reverse order so the first q-projection reuses the
    resident x tiles): the numerator matmul is FOLDED into the output
    projection via M = blockdiag(kv)^T @ Wo^T (per pair: PE transpose +
    2 matmuls, once per rep); the per-token reciprocal of the denominator
    is broadcast 16->128 partitions with a small select matmul and
    multiplied into q ("q'"), so y = q' @ M + b needs exactly one
    projection-shaped GEMM per chunk. q-projection runs `qpre` chunks
    ahead to hide the AllReduce under PE work.
Old f32r v1 (build_program) is kept for reference/ablation only.
"""

import os
import sys

import numpy as np

for _p in ("/opt/trn_rl_repo", "/root/.axon_site/_ro/trn_rl_repo"):
    if os.path.isdir(_p) and _p not in sys.path:
        sys.path.append(_p)

from contextlib import ExitStack

import concourse.bacc as bacc
import concourse.tile as tile
from concourse import mybir
from concourse.alu_op_type import AluOpType
from concourse import bass_utils

F32 = mybir.dt.float32
F32R = mybir.dt.float32r
BF16 = mybir.dt.bfloat16
AF = mybir.ActivationFunctionType

D = 1024        # model dim
H = 16          # heads
HD = 64         # head dim
P = 128         # partitions
NPAIR = 8       # head pairs
KC = 8          # feature chunks of 128
TC = 512        # token chunk
EPS = 1e-6
PW = 129        # pair width in the packed kv|ksum layout


def r(ap):
    return ap.bitcast(F32R)


def build_program(nc, ntok, use_cc=True, reps=1, mmdt="f32r",
                  do_p1=True, do_p2=True, abl=(), fuseq=False, rbc_gps=False,
                  atbf=False, qcp=False, kcp=False):
    abl = set(abl)
    nch = ntok // TC
    MDT = F32R if mmdt == "f32r" else BF16
    XDT = F32 if mmdt == "f32r" else BF16

    def rr(ap):
        return ap.bitcast(F32R) if mmdt == "f32r" else ap

    xt = nc.dram_tensor("xt", [D, ntok], XDT, kind="ExternalInput").ap()
    wqt = nc.dram_tensor("wqt", [D, D], XDT, kind="ExternalInput").ap()
    wkt = nc.dram_tensor("wkt", [D, D], XDT, kind="ExternalInput").ap()
    wvt = nc.dram_tensor("wvt", [D, D], XDT, kind="ExternalInput").ap()
    wot = (nc.dram_tensor("wotb", [D, D], BF16, kind="ExternalInput").ap()
           if atbf else
           nc.dram_tensor("wot", [D, D], XDT, kind="ExternalInput").ap())
    bias = nc.dram_tensor("bias", [1, D], F32, kind="ExternalInput").ap()
    out = nc.dram_tensor("out", [ntok, D], F32, kind="ExternalOutput").ap()

    cc_in = nc.dram_tensor("cc_in", [P, NPAIR * PW], F32).ap()
    cc_out = nc.dram_tensor("cc_out", [P, NPAIR * PW], F32).ap()
    QDT = BF16 if fuseq else MDT
    qsp = nc.dram_tensor("qsp", [D, ntok], BF16).ap() if fuseq else None

    with tile.TileContext(nc) as tc, ExitStack() as es:
        p_w = es.enter_context(tc.tile_pool(name="w", bufs=24 if fuseq else 16))
        p_x = es.enter_context(tc.tile_pool(name="x", bufs=9))
        p_scr = es.enter_context(tc.tile_pool(name="scr", bufs=3 if atbf else 2))
        p_qc = es.enter_context(tc.tile_pool(name="qc", bufs=3))
        p_c1 = es.enter_context(tc.tile_pool(name="c1", bufs=1))
        p_c8 = es.enter_context(tc.tile_pool(name="c8", bufs=8))

        # bias broadcast [1, D] -> [128, D]
        b_row = p_c1.tile([1, D], F32, tag="brow")
        nc.sync.dma_start(b_row[:], bias[:, :])
        bias_bc = p_c1.tile([P, D], F32, tag="bias")
        nc.gpsimd.partition_broadcast(bias_bc[:], b_row[:])

        # select matrices for the reciprocal broadcast (host-prepared constant)
        selc = nc.dram_tensor("selcb" if fuseq else "selc", [H, NPAIR * P],
                          BF16 if fuseq else XDT, kind="ExternalInput").ap()
        sel_t = p_c1.tile([H, NPAIR * P], QDT, tag="sel")
        nc.sync.dma_start(sel_t[:], selc[:, :] if fuseq else rr(selc[:, :]))
        sel = [sel_t[:, p * P:(p + 1) * P] for p in range(NPAIR)]

        # masks for block-diag kv / ksum lhsT construction
        dmask = nc.dram_tensor("dmask", [P, P], F32, kind="ExternalInput").ap()
        dmask_t = p_c1.tile([P, P], F32, tag="dmask")
        nc.sync.dma_start(dmask_t[:], dmask[:, :])
        kmask = nc.dram_tensor("kmask", [P, KC * H], F32,
                               kind="ExternalInput").ap()
        kmask_t = p_c1.tile([P, KC * H], F32, tag="kmask")
        nc.sync.dma_start(kmask_t[:], kmask[:, :])

        # packed kv | ksum accumulator
        kvks = p_c1.tile([P, NPAIR * PW], F32, tag="kvks")
        kvks3 = kvks.rearrange("p (a b) -> p a b", b=PW)

        def load_xt(j):
            ts = []
            for k in range(KC):
                t = p_x.tile([P, TC], MDT, tag="xt")
                nc.sync.dma_start(
                    t[:], rr(xt[k * P:(k + 1) * P, j * TC:(j + 1) * TC]))
                ts.append(t)
            return ts

        for _rep in range(reps):
            # ---- weights (phase 1) ----
            wkt_t = []
            wvt_t = []
            wq1_t = []
            if do_p1:
                for k in range(KC):
                    t = p_w.tile([P, D], MDT, tag="w")
                    nc.sync.dma_start(t[:], rr(wkt[k * P:(k + 1) * P, :]))
                    wkt_t.append(t)
                for k in range(KC):
                    t = p_w.tile([P, D], MDT, tag="w")
                    nc.sync.dma_start(t[:], rr(wvt[k * P:(k + 1) * P, :]))
                    wvt_t.append(t)
                if fuseq:
                    for k in range(KC):
                        t = p_w.tile([P, D], MDT, tag="w")
                        nc.sync.dma_start(t[:], rr(wqt[k * P:(k + 1) * P, :]))
                        wq1_t.append(t)
            nc.vector.memset(kvks[:], 0.0)

            # ================= phase 1: k, v, kv, ksum =================
            with ExitStack() as es1:
                ps_k = es1.enter_context(tc.tile_pool(name="psk", bufs=2, space="PSUM"))
                ps_v = es1.enter_context(tc.tile_pool(name="psv", bufs=2, space="PSUM"))
                ps_kv = es1.enter_context(tc.tile_pool(name="pskv", bufs=2, space="PSUM"))
                p_kphi = es1.enter_context(tc.tile_pool(name="kphi", bufs=6))
                p_v = es1.enter_context(tc.tile_pool(name="vsb", bufs=6))
                if fuseq:
                    ps_q1 = es1.enter_context(
                        tc.tile_pool(name="psq1", bufs=2, space="PSUM"))
                    p_q1 = es1.enter_context(tc.tile_pool(name="q1", bufs=3))

                for j in range(nch if do_p1 else 0):
                    xts = load_xt(j)
                    kphis = []
                    vsbs = []
                    for m in range(4):
                        xm = [xts[k][:, m * P:(m + 1) * P] for k in range(KC)]
                        # k projection + phi -> bf16 [128, 1024]
                        kph = p_kphi.tile([P, D], BF16, tag="kphi")
                        for n in range(2):
                            kp = ps_k.tile([P, TC], F32, tag="kp")
                            for k in range(KC):
                                nc.tensor.matmul(
                                    kp[:], xm[k], wkt_t[k][:, n * TC:(n + 1) * TC],
                                    start=(k == 0), stop=(k == KC - 1))
                            if "phi" in abl:
                                nc.scalar.activation(
                                    kph[:, n * TC:(n + 1) * TC], kp[:], AF.Copy)
                            elif kcp:
                                kc_ = p_qc.tile([P, TC], F32, tag="kc")
                                nc.scalar.activation(kc_[:], kp[:], AF.Copy)
                                mx = p_scr.tile([P, TC], F32, tag="mx")
                                nc.vector.tensor_scalar_min(mx[:], kc_[:], 0.0)
                                ex = p_scr.tile([P, TC], F32, tag="ex")
                                nc.scalar.activation(ex[:], mx[:], AF.Exp)
                                nc.vector.scalar_tensor_tensor(
                                    kph[:, n * TC:(n + 1) * TC], kc_[:], 1.0,
                                    ex[:], AluOpType.add, AluOpType.max)
                            else:
                                mx = p_scr.tile([P, TC], F32, tag="mx")
                                nc.vector.tensor_scalar_min(mx[:], kp[:], 0.0)
                                ex = p_scr.tile([P, TC], F32, tag="ex")
                                nc.scalar.activation(ex[:], mx[:], AF.Exp)
                                nc.vector.scalar_tensor_tensor(
                                    kph[:, n * TC:(n + 1) * TC], kp[:], 1.0, ex[:],
                                    AluOpType.add, AluOpType.max)
                        kphis.append(kph)
                        # v projection -> bf16 [128, 1032] interleaved with ones cols
                        vsb = p_v.tile([P, NPAIR * PW], BF16, tag="vsb")
                        vsb3 = vsb.rearrange("p (a b) -> p a b", b=PW)
                        for n in range(2):
                            vp = ps_v.tile([P, TC], F32, tag="vp")
                            for k in range(KC):
                                nc.tensor.matmul(
                                    vp[:], xm[k], wvt_t[k][:, n * TC:(n + 1) * TC],
                                    start=(k == 0), stop=(k == KC - 1))
                            if "vcopy" not in abl:
                                nc.scalar.activation(
                                    vsb3[:, 4 * n:4 * n + 4, 0:P],
                                    vp.rearrange("p (a b) -> p a b", b=P), AF.Copy)
                        if "vcopy" not in abl:
                            nc.vector.memset(vsb3[:, :, P:PW], 1.0)
                        else:
                            nc.vector.memset(vsb[:], 1.0)
                        vsbs.append(vsb)
                    # fused q projection + phi + spill to DRAM (bf16)
                    if fuseq:
                        for c in range(KC):
                            qp1 = ps_q1.tile([P, TC], F32, tag="qp1")
                            for k in range(KC):
                                nc.tensor.matmul(
                                    qp1[:], wq1_t[k][:, c * P:(c + 1) * P],
                                    xts[k][:],
                                    start=(k == 0), stop=(k == KC - 1))
                            q1 = p_q1.tile([P, TC], BF16, tag="q1")
                            if "phi" in abl:
                                nc.scalar.activation(q1[:], qp1[:], AF.Copy)
                            else:
                                mx = p_scr.tile([P, TC], F32, tag="mx")
                                nc.vector.tensor_scalar_min(mx[:], qp1[:], 0.0)
                                ex = p_scr.tile([P, TC], F32, tag="ex")
                                nc.scalar.activation(ex[:], mx[:], AF.Exp)
                                nc.vector.scalar_tensor_tensor(
                                    q1[:], qp1[:], 1.0, ex[:], AluOpType.add,
                                    AluOpType.max)
                            nc.sync.dma_start(
                                qsp[c * P:(c + 1) * P, j * TC:(j + 1) * TC],
                                q1[:])
                    # kv + ksum accumulation per pair
                    for p in range(NPAIR if "kv" not in abl else 0):
                        kvp = ps_kv.tile([P, PW], F32, tag="kvp")
                        for m in range(4):
                            nc.tensor.matmul(
                                kvp[:], kphis[m][:, p * P:(p + 1) * P],
                                vsbs[m].rearrange("p (a b) -> p a b", b=PW)[:, p, :],
                                start=(m == 0), stop=(m == 3))
                        nc.vector.tensor_tensor(
                            kvks3[:, p, :], kvp[:], kvks3[:, p, :], AluOpType.add)

            # ================= allreduce within pairs =================
            nc.sync.dma_start(cc_in[:, :], kvks[:])
            if use_cc:
                nc.gpsimd.collective_compute(
                    "AllReduce", AluOpType.add,
                    replica_groups=[[0, 1], [2, 3], [4, 5], [6, 7]],
                    ins=[cc_in[:, :]], outs=[cc_out[:, :]])
            else:
                nc.sync.dma_start(cc_out[:, :], cc_in[:, :])
            red = p_c1.tile([P, NPAIR * PW], F32, tag="red")
            nc.sync.dma_start(red[:], cc_out[:, :])
            red3 = red.rearrange("p (a b) -> p a b", b=PW)

            # block-diag kv lhsT per pair (diag-mask multiply), block ksum lhsT
            # per chunk (per-partition scalar multiply against a column mask).
            kvbd = []
            ksbd = []
            for p in range(NPAIR):
                t = p_c8.tile([P, P], QDT, tag="kvbd")
                nc.vector.tensor_tensor(t[:], red3[:, p, 0:P], dmask_t[:],
                                        AluOpType.mult)
                kvbd.append(t)
            for c in range(KC):
                t = p_c8.tile([P, H], QDT, tag="ksbd")
                nc.vector.tensor_scalar(
                    t[:], kmask_t[:, c * H:(c + 1) * H], red3[:, c, P:PW], None,
                    AluOpType.mult)
                ksbd.append(t)

            # ---- weights (phase 2) ----
            wqt_t = []
            wot_t = []
            if do_p2:
                for k in range(KC):
                    t = p_w.tile([P, D], MDT, tag="w")
                    nc.sync.dma_start(t[:], rr(wqt[k * P:(k + 1) * P, :]))
                    wqt_t.append(t)
                for k in range(KC):
                    if atbf:
                        t = p_w.tile([P, D], BF16, tag="w")
                        nc.sync.dma_start(t[:], wot[k * P:(k + 1) * P, :])
                    else:
                        t = p_w.tile([P, D], MDT, tag="w")
                        nc.sync.dma_start(t[:], rr(wot[k * P:(k + 1) * P, :]))
                    wot_t.append(t)

            # ================= phase 2: q, numerator, denom, y =================
            with ExitStack() as es2:
                ps_q = es2.enter_context(tc.tile_pool(name="psq", bufs=2, space="PSUM"))
                ps_num = es2.enter_context(tc.tile_pool(name="psnum", bufs=2, space="PSUM"))
                ps_rbc = es2.enter_context(tc.tile_pool(name="psrbc", bufs=2, space="PSUM"))
                ps_y = es2.enter_context(tc.tile_pool(name="psy", bufs=2, space="PSUM"))
                p_qt = es2.enter_context(tc.tile_pool(name="qt", bufs=16))
                p_at = es2.enter_context(tc.tile_pool(name="at", bufs=16 if atbf else 11))
                p_rbc = es2.enter_context(tc.tile_pool(name="rbc", bufs=3))
                p_y = es2.enter_context(tc.tile_pool(name="ysb", bufs=3))
                p_dn = es2.enter_context(tc.tile_pool(name="dn", bufs=1))

                def emit_qproj(j):
                    xts = load_xt(j)
                    qts = []
                    for c in range(KC):
                        qp = ps_q.tile([P, TC], F32, tag="qp")
                        for k in range(KC):
                            nc.tensor.matmul(
                                qp[:], wqt_t[k][:, c * P:(c + 1) * P], xts[k][:],
                                start=(k == 0), stop=(k == KC - 1))
                        qt_ = p_qt.tile([P, TC], MDT, tag="qt")
                        if "phi" in abl:
                            nc.scalar.activation(qt_[:], qp[:], AF.Copy)
                        elif qcp:
                            # release the PSUM tile with one ACT copy, then
                            # run the phi chain from SBUF so the PE is not
                            # gated on the full DVE/ACT consumer chain
                            qc = p_qc.tile([P, TC], F32, tag="qc")
                            nc.scalar.activation(qc[:], qp[:], AF.Copy)
                            mx = p_scr.tile([P, TC], F32, tag="mx")
                            nc.vector.tensor_scalar_min(mx[:], qc[:], 0.0)
                            ex = p_scr.tile([P, TC], F32, tag="ex")
                            nc.scalar.activation(ex[:], mx[:], AF.Exp)
                            nc.vector.scalar_tensor_tensor(
                                qt_[:], qc[:], 1.0, ex[:], AluOpType.add,
                                AluOpType.max)
                        else:
                            mx = p_scr.tile([P, TC], F32, tag="mx")
                            nc.vector.tensor_scalar_min(mx[:], qp[:], 0.0)
                            ex = p_scr.tile([P, TC], F32, tag="ex")
                            nc.scalar.activation(ex[:], mx[:], AF.Exp)
                            nc.vector.scalar_tensor_tensor(
                                qt_[:], qp[:], 1.0, ex[:], AluOpType.add,
                                AluOpType.max)
                        qts.append(qt_)
                    return qts

                def emit_y(j, attns):
                    if "ycompute" in abl:
                        return
                    for m in range(4):
                        for n in range(2):
                            yp = ps_y.tile([P, TC], F32, tag="yp")
                            for k in range(KC):
                                nc.tensor.matmul(
                                    yp[:], attns[k][:, m * P:(m + 1) * P],
                                    wot_t[k][:, n * TC:(n + 1) * TC],
                                    start=(k == 0), stop=(k == KC - 1))
                            ysb = p_y.tile([P, TC], F32, tag="ysb")
                            nc.vector.tensor_tensor(
                                ysb[:], yp[:], bias_bc[:, n * TC:(n + 1) * TC],
                                AluOpType.add)
                            row0 = j * TC + m * P
                            nc.sync.dma_start(
                                out[row0:row0 + P, n * TC:(n + 1) * TC], ysb[:])

                def load_qt(j):
                    ts = []
                    for c in range(KC):
                        t = p_x.tile([P, TC], BF16, tag="qtl")
                        nc.sync.dma_start(
                            t[:], qsp[c * P:(c + 1) * P, j * TC:(j + 1) * TC])
                        ts.append(t)
                    return ts

                if fuseq:
                    qtiles = {0: load_qt(0)} if do_p2 else {}
                else:
                    qtiles = {0: emit_qproj(0)} if do_p2 else {}
                attn_prev = None
                for j in range(nch if do_p2 else 0):
                    if j + 1 < nch:
                        qtiles[j + 1] = load_qt(j + 1) if fuseq else emit_qproj(j + 1)
                    qts = qtiles.pop(j)
                    if "numrbc" in abl:
                        attn_prev_new = qts
                        if attn_prev is not None:
                            emit_y(j - 1, attn_prev)
                        attn_prev = attn_prev_new
                        continue
                    # denominator for all heads: [16, TC]
                    dps_full = ps_rbc.tile([P, TC], F32, tag="rbps")
                    dps = dps_full[0:H, :]
                    for c in range(KC):
                        nc.tensor.matmul(dps[:], ksbd[c][:], qts[c][:],
                                         start=(c == 0), stop=(c == KC - 1))
                    if attn_prev is not None:
                        emit_y(j - 1, attn_prev)
                    dsb = p_dn.tile([H, TC], F32, tag="dsb")
                    nc.vector.tensor_scalar_add(dsb[:], dps[:], EPS)
                    rcp = p_dn.tile([H, TC], F32 if rbc_gps else QDT, tag="rcp")
                    with nc.allow_low_precision(reason="f32r rounding of recip"):
                        nc.vector.reciprocal(rcp[:], dsb[:])
                    attns = []
                    for p in range(NPAIR):
                        np_ = ps_num.tile([P, TC], F32, tag="nps")
                        nc.tensor.matmul(np_[:], kvbd[p][:], qts[p][:],
                                         start=True, stop=True)
                        rbs = p_rbc.tile([P, TC], F32, tag="rbs")
                        if rbc_gps:
                            nc.gpsimd.partition_broadcast(
                                rbs[0:HD, :], rcp[2 * p:2 * p + 1, :])
                            nc.gpsimd.partition_broadcast(
                                rbs[HD:P, :], rcp[2 * p + 1:2 * p + 2, :])
                        else:
                            rb = ps_rbc.tile([P, TC], F32, tag="rbps")
                            nc.tensor.matmul(rb[:], sel[p], rcp[:],
                                             start=True, stop=True)
                            nc.scalar.activation(rbs[:], rb[:], AF.Copy)
                        at = p_at.tile([P, TC], BF16 if atbf else MDT, tag="at")
                        nc.vector.tensor_tensor(at[:], np_[:], rbs[:], AluOpType.mult)
                        attns.append(at)
                    attn_prev = attns
                if do_p2:
                    emit_y(nch - 1, attn_prev)

    return nc


def build_program2(nc, ntok, reps=1, use_cc=True, do_p1=True, do_p2=True,
                   abl=(), qpre=3, ccbf=True, rev=True, ydma_act=False,
                   batch=True, scsel=False, phimode="qc"):
    """v2: all-bf16 matmuls; numerator folded into the output projection via
    M = blockdiag(kv) @ Wo^T; reciprocal applied to q via Pool broadcast;
    all four weight matrices prefetched at rep start (bf16 halves SBUF and
    DMA so they fit resident simultaneously); q-projection runs `qpre`
    chunks ahead so the pair AllReduce hides behind PE work.
    """
    abl = set(abl)
    nch = ntok // TC

    xt = nc.dram_tensor("xt", [D, ntok], BF16, kind="ExternalInput").ap()
    wkb = nc.dram_tensor("wkb", [D, D], BF16, kind="ExternalInput").ap()
    wvb = nc.dram_tensor("wvb", [D, D], BF16, kind="ExternalInput").ap()
    wqb = nc.dram_tensor("wqb", [D, D], BF16, kind="ExternalInput").ap()
    wob = nc.dram_tensor("wob", [D, D], BF16, kind="ExternalInput").ap()
    bias = nc.dram_tensor("bias", [1, D], F32, kind="ExternalInput").ap()
    ident = nc.dram_tensor("ident", [P, P], BF16, kind="ExternalInput").ap()
    selc = nc.dram_tensor("selc", [H, NPAIR * P], BF16,
                          kind="ExternalInput").ap()
    dmask = nc.dram_tensor("dmask", [P, P], F32, kind="ExternalInput").ap()
    kmask = nc.dram_tensor("kmask", [P, KC * H], F32,
                           kind="ExternalInput").ap()
    out = nc.dram_tensor("out", [ntok, D], F32, kind="ExternalOutput").ap()

    CDT = BF16 if ccbf else F32
    cc_in = nc.dram_tensor("cc_in", [P, NPAIR * PW], CDT).ap()
    cc_out = nc.dram_tensor("cc_out", [P, NPAIR * PW], CDT).ap()

    with tile.TileContext(nc) as tc, ExitStack() as es:
        p_w = es.enter_context(tc.tile_pool(name="w", bufs=4 if batch else 32))
        p_x = es.enter_context(tc.tile_pool(name="x", bufs=2 if batch else 9))
        p_scr = es.enter_context(tc.tile_pool(name="scr", bufs=2))
        p_qt = es.enter_context(tc.tile_pool(name="qt", bufs=32))
        p_kphi = es.enter_context(tc.tile_pool(name="kphi", bufs=5))
        p_v = es.enter_context(tc.tile_pool(name="vsb", bufs=5))
        p_m = es.enter_context(tc.tile_pool(name="m", bufs=8))
        p_kv8 = es.enter_context(tc.tile_pool(name="kv8", bufs=8))
        p_rbs = es.enter_context(tc.tile_pool(name="rbs", bufs=4))
        p_dn = es.enter_context(
            tc.tile_pool(name="dn", bufs=2 if scsel else 1))
        if not scsel:
            p_rfl = es.enter_context(tc.tile_pool(name="rfl", bufs=1))
        p_y = es.enter_context(
            tc.tile_pool(name="ysb", bufs=2 if batch else 3))
        p_c1 = es.enter_context(tc.tile_pool(name="c1", bufs=1))

        xt3 = xt.rearrange("(a p) n -> p a n", p=P)
        wk3 = wkb.rearrange("(a p) d -> p a d", p=P)
        wv3 = wvb.rearrange("(a p) d -> p a d", p=P)
        wq3 = wqb.rearrange("(a p) d -> p a d", p=P)
        wo3 = wob.rearrange("(a p) d -> p a d", p=P)

        # constants
        b_row = p_c1.tile([1, D], F32, tag="brow")
        nc.sync.dma_start(b_row[:], bias[:, :])
        bias_bc = p_c1.tile([P, D], F32, tag="bias")
        nc.gpsimd.partition_broadcast(bias_bc[:], b_row[:])
        ident_t = p_c1.tile([P, P], BF16, tag="ident")
        nc.sync.dma_start(ident_t[:], ident[:, :])
        if scsel:
            sel_t = p_c1.tile([H, NPAIR * P], BF16, tag="sel")
            nc.sync.dma_start(sel_t[:], selc[:, :])
            sel = [sel_t[:, p * P:(p + 1) * P] for p in range(NPAIR)]
        dmask_t = p_c1.tile([P, P], F32, tag="dmask")
        nc.sync.dma_start(dmask_t[:], dmask[:, :])
        kmask_t = p_c1.tile([P, KC * H], F32, tag="kmask")
        nc.sync.dma_start(kmask_t[:], kmask[:, :])

        kvks = p_c1.tile([P, NPAIR * PW], F32, tag="kvks")
        kvks3 = kvks.rearrange("p (a b) -> p a b", b=PW)

        def load_w(src3, src2):
            if batch:
                t = p_w.tile([P, KC * D], BF16, tag="w")
                t3 = t.rearrange("p (a d) -> p a d", d=D)
                nc.sync.dma_start(t3[:, :, :], src3[:, :, :])
                return [t3[:, k, :] for k in range(KC)]
            ts = []
            for k in range(KC):
                t = p_w.tile([P, D], BF16, tag="w")
                nc.sync.dma_start(t[:], src2[k * P:(k + 1) * P, :])
                ts.append(t)
            return ts

        def load_xt(j):
            if batch:
                t = p_x.tile([P, KC * TC], BF16, tag="xt")
                t3 = t.rearrange("p (a n) -> p a n", n=TC)
                nc.sync.dma_start(
                    t3[:, :, :], xt3[:, :, j * TC:(j + 1) * TC])
                return [t3[:, k, :] for k in range(KC)]
            ts = []
            for k in range(KC):
                t = p_x.tile([P, TC], BF16, tag="xt")
                nc.sync.dma_start(
                    t[:], xt[k * P:(k + 1) * P, j * TC:(j + 1) * TC])
                ts.append(t)
            return ts

        def phi_from(psum, dst):
            # dst = max(src + 1, exp(min(src, 0)))
            src = psum
            if phimode == "qc":
                qc = p_scr.tile([P, TC], BF16, tag="qc")
                nc.scalar.activation(qc[:], psum[:], AF.Copy)
                src = qc
            mx = p_scr.tile([P, TC], BF16, tag="mx")
            if phimode == "pool":
                nc.gpsimd.tensor_scalar_min(mx[:], src[:], 0.0)
            else:
                nc.vector.tensor_scalar_min(mx[:], src[:], 0.0)
            ex = p_scr.tile([P, TC], BF16, tag="ex")
            nc.scalar.activation(ex[:], mx[:], AF.Exp)
            nc.vector.scalar_tensor_tensor(
                dst, src[:], 1.0, ex[:], AluOpType.add, AluOpType.max)

        for _rep in range(reps):
            if do_p1 and not batch:
                # interleave wk with x(0) so the first k-proj group is
                # DMA-paced rather than blocked on the full weight prefetch
                wkt = []
                xts0 = []
                for k in range(KC):
                    t = p_w.tile([P, D], BF16, tag="w")
                    nc.sync.dma_start(t[:], wkb[k * P:(k + 1) * P, :])
                    wkt.append(t)
                    tx = p_x.tile([P, TC], BF16, tag="xt")
                    nc.sync.dma_start(tx[:], xt[k * P:(k + 1) * P, 0:TC])
                    xts0.append(tx)
            elif do_p1:
                wkt = load_w(wk3, wkb)
                xts0 = load_xt(0)
            else:
                wkt, xts0 = [], []
            wvt = load_w(wv3, wvb) if do_p1 else []
            wqt = load_w(wq3, wqb) if do_p2 else []
            wot = load_w(wo3, wob) if do_p2 else []
            nc.vector.memset(kvks[:], 0.0)

            # ================= phase 1: k, v, kv|ksum =================
            with ExitStack() as es1:
                ps_k = es1.enter_context(
                    tc.tile_pool(name="psk", bufs=2, space="PSUM"))
                ps_v = es1.enter_context(
                    tc.tile_pool(name="psv", bufs=2, space="PSUM"))
                ps_kv = es1.enter_context(
                    tc.tile_pool(name="pskv", bufs=2, space="PSUM"))

                for j in range(nch if do_p1 else 0):
                    xts = xts0 if j == 0 else load_xt(j)
                    kphis = []
                    vsbs = []
                    for m in range(4):
                        xm = [xts[k][:, m * P:(m + 1) * P] for k in range(KC)]
                        kph = p_kphi.tile([P, D], BF16, tag="kphi")
                        for n in range(2):
                            kp = ps_k.tile([P, TC], F32, tag="kp")
                            for k in range(KC):
                                nc.tensor.matmul(
                                    kp[:], xm[k],
                                    wkt[k][:, n * TC:(n + 1) * TC],
                                    start=(k == 0), stop=(k == KC - 1))
                            if "phi" in abl:
                                nc.scalar.activation(
                                    kph[:, n * TC:(n + 1) * TC], kp[:],
                                    AF.Copy)
                            else:
                                phi_from(kp, kph[:, n * TC:(n + 1) * TC])
                        kphis.append(kph)
                        vsb = p_v.tile([P, NPAIR * PW], BF16, tag="vsb")
                        vsb3 = vsb.rearrange("p (a b) -> p a b", b=PW)
                        for n in range(2):
                            vp = ps_v.tile([P, TC], F32, tag="vp")
                            for k in range(KC):
                                nc.tensor.matmul(
                                    vp[:], xm[k],
                                    wvt[k][:, n * TC:(n + 1) * TC],
                                    start=(k == 0), stop=(k == KC - 1))
                            nc.scalar.activation(
                                vsb3[:, 4 * n:4 * n + 4, 0:P],
                                vp.rearrange("p (a b) -> p a b", b=P),
                                AF.Copy)
                        nc.vector.memset(vsb3[:, :, P:PW], 1.0)
                        vsbs.append(vsb)
                    for p in range(NPAIR if "kv" not in abl else 0):
                        kvp = ps_kv.tile([P, PW], F32, tag="kvp")
                        for m in range(4):
                            nc.tensor.matmul(
                                kvp[:], kphis[m][:, p * P:(p + 1) * P],
                                vsbs[m].rearrange(
                                    "p (a b) -> p a b", b=PW)[:, p, :],
                                start=(m == 0), stop=(m == 3))
                        nc.vector.tensor_tensor(
                            kvks3[:, p, :], kvp[:], kvks3[:, p, :],
                            AluOpType.add)

            # ================= allreduce within pairs =================
            if ccbf:
                kvksb = p_c1.tile([P, NPAIR * PW], BF16, tag="kvksb")
                nc.vector.tensor_copy(kvksb[:], kvks[:])
                nc.sync.dma_start(cc_in[:, :], kvksb[:])
            else:
                nc.sync.dma_start(cc_in[:, :], kvks[:])
            if use_cc:
                nc.gpsimd.collective_compute(
                    "AllReduce", AluOpType.add,
                    replica_groups=[[0, 1], [2, 3], [4, 5], [6, 7]],
                    ins=[cc_in[:, :]], outs=[cc_out[:, :]])
            else:
                nc.sync.dma_start(cc_out[:, :], cc_in[:, :])
            red = p_c1.tile([P, NPAIR * PW], CDT, tag="red")
            nc.sync.dma_start(red[:], cc_out[:, :])
            red3 = red.rearrange("p (a b) -> p a b", b=PW)

            # ================= phase 2: q, den, y = q' @ M =================
            with ExitStack() as es2:
                ps_q = es2.enter_context(
                    tc.tile_pool(name="psq", bufs=2, space="PSUM"))
                ps_dn = es2.enter_context(
                    tc.tile_pool(name="psdn", bufs=1 if scsel else 2,
                                 space="PSUM"))
                ps_y = es2.enter_context(
                    tc.tile_pool(name="psy", bufs=2, space="PSUM"))
                mstack = ExitStack()
                ps_mt = mstack.enter_context(
                    tc.tile_pool(name="psmt", bufs=1, space="PSUM"))

                def emit_qproj(j, xts=None):
                    if xts is None:
                        xts = load_xt(j)
                    qts = []
                    for c in range(KC):
                        qp = ps_q.tile([P, TC], F32, tag="qp")
                        for k in range(KC):
                            nc.tensor.matmul(
                                qp[:], wqt[k][:, c * P:(c + 1) * P], xts[k][:],
                                start=(k == 0), stop=(k == KC - 1))
                        qt_ = p_qt.tile([P, TC], BF16, tag="qt")
                        if "phi" in abl:
                            nc.scalar.activation(qt_[:], qp[:], AF.Copy)
                        else:
                            phi_from(qp, qt_[:])
                        qts.append(qt_)
                    return qts

                def emit_den(j, qts):
                    # denominator for all heads -> reciprocal on partition 0
                    dps_full = ps_dn.tile([P, TC], F32, tag="dn")
                    dps = dps_full[0:H, :]
                    for c in range(KC):
                        nc.tensor.matmul(dps[:], ksbd[c][:], qts[c][:],
                                         start=(c == 0), stop=(c == KC - 1))
                    dsb = p_dn.tile([H, TC], F32, tag="dsb")
                    nc.vector.tensor_scalar_add(dsb[:], dps[:], EPS)
                    rcp = p_dn.tile([H, TC], BF16, tag="rcp")
                    with nc.allow_low_precision(reason="bf16 recip"):
                        nc.vector.reciprocal(rcp[:], dsb[:])
                    if scsel:
                        return rcp
                    rfl = p_rfl.tile([1, H * TC], BF16, tag="rfl")
                    rfl3 = rfl.rearrange("p (a b) -> p a b", b=TC)
                    nc.sync.dma_start(rfl3[:, :, :], rcp[:, :])
                    if "scale" in abl:
                        return rcp
                    for c in range(KC):
                        rbe = p_rbs.tile([P, TC], BF16, tag="rbs")
                        nc.gpsimd.partition_broadcast(
                            rbe[:, :], rfl3[:, 2 * c, :])
                        rbo = p_rbs.tile([P, TC], BF16, tag="rbs")
                        nc.gpsimd.partition_broadcast(
                            rbo[:, :], rfl3[:, 2 * c + 1, :])
                        nc.vector.tensor_tensor(
                            qts[c][0:HD, :], qts[c][0:HD, :], rbe[0:HD, :],
                            AluOpType.mult)
                        nc.vector.tensor_tensor(
                            qts[c][HD:P, :], qts[c][HD:P, :], rbo[HD:P, :],
                            AluOpType.mult)
                    return rcp

                def emit_scale(j, qts, rcp):
                    # q' = q * rcp via PE select-matmul broadcast
                    if not scsel or "scale" in abl:
                        return
                    for c in range(KC):
                        rb = ps_rb.tile([P, TC], F32, tag="rb")
                        nc.tensor.matmul(rb[:], sel[c], rcp[:],
                                         start=True, stop=True)
                        rbs = p_rbs.tile([P, TC], BF16, tag="rbs")
                        nc.scalar.activation(rbs[:], rb[:], AF.Copy)
                        nc.vector.tensor_tensor(
                            qts[c][:, :], qts[c][:, :], rbs[:, :],
                            AluOpType.mult)

                def emit_y(j, qts):
                    if "y" in abl:
                        return
                    eng = nc.scalar if ydma_act else nc.sync
                    for m in range(4):
                        row0 = j * TC + m * P
                        if batch:
                            ysb = p_y.tile([P, D], F32, tag="ysb")
                        else:
                            ysb = None
                        for n in range(2):
                            yp = ps_y.tile([P, TC], F32, tag="yp")
                            for c in range(KC):
                                nc.tensor.matmul(
                                    yp[:], qts[c][:, m * P:(m + 1) * P],
                                    m_t[c][:, n * TC:(n + 1) * TC],
                                    start=(c == 0), stop=(c == KC - 1))
                            if batch:
                                nc.vector.tensor_tensor(
                                    ysb[:, n * TC:(n + 1) * TC], yp[:],
                                    bias_bc[:, n * TC:(n + 1) * TC],
                                    AluOpType.add)
                            else:
                                ysb = p_y.tile([P, TC], F32, tag="ysb")
                                nc.vector.tensor_tensor(
                                    ysb[:], yp[:],
                                    bias_bc[:, n * TC:(n + 1) * TC],
                                    AluOpType.add)
                                eng.dma_start(
                                    out[row0:row0 + P,
                                        n * TC:(n + 1) * TC], ysb[:])
                        if batch:
                            eng.dma_start(out[row0:row0 + P, :], ysb[:])

                if do_p2:
                    # reversed chunk order reuses phase-1's resident x(last)
                    order = list(range(nch))[::-1] if rev else list(range(nch))
                    qtiles = {}
                    for idx in range(min(qpre, nch)):
                        j = order[idx]
                        xts_j = None
                        if rev and do_p1 and idx == 0:
                            xts_j = xts
                        qtiles[j] = emit_qproj(j, xts_j)

                    # M = blockdiag(kv) @ Wo^T  (per pair: transpose + 2 mm)
                    m_t = []
                    ksbd = []
                    for p in range(NPAIR):
                        kvbd = p_kv8.tile([P, P], BF16, tag="kvbd")
                        nc.vector.tensor_tensor(
                            kvbd[:], red3[:, p, 0:P], dmask_t[:],
                            AluOpType.mult)
                        tp = ps_mt.tile([P, P], BF16, tag="tp")
                        nc.tensor.transpose(tp[:], kvbd[:], ident_t[:])
                        kvt = p_kv8.tile([P, P], BF16, tag="kvt")
                        nc.scalar.activation(kvt[:], tp[:], AF.Copy)
                        mt = p_m.tile([P, D], BF16, tag="m")
                        for n in range(2):
                            mm = ps_mt.tile([P, TC], F32, tag="mm")
                            nc.tensor.matmul(mm[:], kvt[:],
                                             wot[p][:, n * TC:(n + 1) * TC],
                                             start=True, stop=True)
                            nc.scalar.activation(
                                mt[:, n * TC:(n + 1) * TC], mm[:], AF.Copy)
                        m_t.append(mt)
                        t = p_kv8.tile([P, H], BF16, tag="ksbd")
                        if ccbf:
                            ksf = p_kv8.tile([P, 1], F32, tag="ksf")
                            nc.vector.tensor_copy(ksf[:], red3[:, p, P:PW])
                            nc.vector.tensor_scalar(
                                t[:], kmask_t[:, p * H:(p + 1) * H],
                                ksf[:], None, AluOpType.mult)
                        else:
                            nc.vector.tensor_scalar(
                                t[:], kmask_t[:, p * H:(p + 1) * H],
                                red3[:, p, P:PW], None, AluOpType.mult)
                        ksbd.append(t)
                    mstack.close()
                    if scsel:
                        ps_rb = es2.enter_context(
                            tc.tile_pool(name="psrb", bufs=2, space="PSUM"))

                    rcps = {order[0]: emit_den(order[0], qtiles[order[0]])}
                    for idx, j in enumerate(order):
                        if idx + 1 < nch:
                            jn = order[idx + 1]
                            rcps[jn] = emit_den(jn, qtiles[jn])
                        emit_scale(j, qtiles[j], rcps.pop(j))
                        emit_y(j, qtiles.pop(j))
                        if idx + qpre < nch:
                            jq = order[idx + qpre]
                            qtiles[jq] = emit_qproj(jq)
                else:
                    mstack.close()

    return nc


def make_in_maps2(x, Wq, Wk, Wv, Wo, bo, n_cores=8):
    import ml_dtypes
    bf = ml_dtypes.bfloat16
    x = np.asarray(x, dtype=np.float32)
    B, N, _ = x.shape
    npc = B * N // n_cores
    halves = N // npc
    wkb = np.ascontiguousarray(np.asarray(Wk, np.float32).T).astype(bf)
    wvb = np.ascontiguousarray(np.asarray(Wv, np.float32).T).astype(bf)
    wqb = np.ascontiguousarray(np.asarray(Wq, np.float32).T).astype(bf)
    wob = np.ascontiguousarray(np.asarray(Wo, np.float32).T).astype(bf)
    b_ = np.asarray(bo, np.float32).reshape(1, D)
    ident = np.eye(P, dtype=bf)
    selc = np.zeros((H, NPAIR * P), dtype=bf)
    for p in range(NPAIR):
        selc[2 * p, p * P:p * P + HD] = 1.0
        selc[2 * p + 1, p * P + HD:(p + 1) * P] = 1.0
    dmask = np.zeros((P, P), dtype=np.float32)
    dmask[:HD, :HD] = 1.0
    dmask[HD:, HD:] = 1.0
    kmask = np.zeros((P, KC * H), dtype=np.float32)
    for c in range(KC):
        kmask[:HD, c * H + 2 * c] = 1.0
        kmask[HD:, c * H + 2 * c + 1] = 1.0
    in_maps = []
    for i in range(n_cores):
        b, h = divmod(i, halves)
        xs = np.ascontiguousarray(x[b, h * npc:(h + 1) * npc, :].T).astype(bf)
        in_maps.append({"xt": xs, "wkb": wkb, "wvb": wvb, "wqb": wqb,
                        "wob": wob, "bias": b_, "ident": ident, "selc": selc,
                        "dmask": dmask, "kmask": kmask})
    return in_maps, npc


last_result = None


def build_compiled(ntok, n_cores=8):
    nc = bacc.Bacc("TRN2", target_bir_lowering=False, debug=False,
                   num_devices=n_cores)
    build_program2(nc, ntok)
    nc.compile()
    from concourse.bass_interp import get_hw_module
    nc.m = get_hw_module(nc.m)
    return nc


def _run(in_maps, ntok, n_cores=8):
    # NTFF tracing is unsupported under this axon client; make sure the
    # spmd runner never takes the trace path.
    os.environ["BASS_NEVER_TRACE"] = "1"
    key = (ntok, n_cores)
    if key not in _prog_cache:
        _prog_cache[key] = build_compiled(ntok, n_cores)
    nc = _prog_cache[key]
    res = bass_utils.run_bass_kernel_spmd(nc, in_maps, list(range(n_cores)))
    global last_result
    last_result = res
    return res


def make_in_maps(x, Wq, Wk, Wv, Wo, bo, n_cores=8, mmdt="f32r", fuseq=False):
    import ml_dtypes
    if mmdt == "f32r":
        xdt = np.float32
    else:
        xdt = ml_dtypes.bfloat16
    sdt = ml_dtypes.bfloat16 if fuseq else xdt
    x = np.asarray(x, dtype=np.float32)
    B, N, _ = x.shape
    npc = B * N // n_cores  # tokens per core
    halves = N // npc       # token halves per batch item
    wqt = np.ascontiguousarray(np.asarray(Wq, np.float32).T).astype(xdt)
    wkt = np.ascontiguousarray(np.asarray(Wk, np.float32).T).astype(xdt)
    wvt = np.ascontiguousarray(np.asarray(Wv, np.float32).T).astype(xdt)
    wot = np.ascontiguousarray(np.asarray(Wo, np.float32).T).astype(xdt)
    b_ = np.asarray(bo, np.float32).reshape(1, D)
    selc = np.zeros((H, NPAIR * P), dtype=sdt)
    for p in range(NPAIR):
        selc[2 * p, p * P:p * P + HD] = 1.0
        selc[2 * p + 1, p * P + HD:(p + 1) * P] = 1.0
    dmask = np.zeros((P, P), dtype=np.float32)
    dmask[:HD, :HD] = 1.0
    dmask[HD:, HD:] = 1.0
    kmask = np.zeros((P, KC * H), dtype=np.float32)
    for c in range(KC):
        kmask[:HD, c * H + 2 * c] = 1.0
        kmask[HD:, c * H + 2 * c + 1] = 1.0
    in_maps = []
    for i in range(n_cores):
        b, h = divmod(i, halves)
        xs = np.ascontiguousarray(x[b, h * npc:(h + 1) * npc, :].T).astype(xdt)
        key = "selcb" if fuseq else "selc"
        in_maps.append({"xt": xs, "wqt": wqt, "wkt": wkt, "wvt": wvt,
                        "wot": wot, "bias": b_, key: selc,
                        "dmask": dmask, "kmask": kmask})
    return in_maps, npc


_prog_cache = {}


def kernel(x, Wq, Wk, Wv, Wo, bo):
    x = np.asarray(x, dtype=np.float32)
    B, N, _ = x.shape
    n_cores = 8
    in_maps, npc = make_in_maps2(x, Wq, Wk, Wv, Wo, bo, n_cores)
    halves = N // npc
    res = _run(in_maps, npc, n_cores)
    out = np.empty((B, N, D), dtype=np.float32)
    for i in range(n_cores):
        b, h = divmod(i, halves)
        out[b, h * npc:(h + 1) * npc, :] = res.results[i]["out"]
    return out

